# revision 1
# baseline (speedup 1.0000x reference)
"""MoE gate routing kernel for Trainium2 (8 NeuronCores, SPMD token-parallel).

Problem: hidden_states [4,4096,4096] f32, weight [256,4096] f32, bias [256] f32.
reference: logits = hs @ W.T; scores = sigmoid(logits); grouped top-2-sum group
scores -> top-4 groups -> top-8 experts; returns (topk_idx int32 [n,8],
topk_weight f32 [n,8]) with weights = normalized sigmoid scores * 2.5.

Sharding: token dim (n = 16384) split across 8 cores (2048 tokens each); the
gate weight and bias are replicated.

Split-precision GEMM (all splits and the transpose happen on host):
  hs = hi (fp16) + lo,  w = whi (fp16) + wlo
  logits = hi@whi  (P1: fp16 matmuls, fp32 PSUM accumulate — exact products)
         + 2^-18 * [ (lo*2^12)@(w*2^6) + hi@(wlo*2^18) ]   (P2)
  P2 runs as ONE fp8e4m3 DoubleRow pass: stationary pairs [lo8; hi8],
  moving pairs [w8; wlo8] — two K-slots per PE cell at 0.5 cycles/row.
  hi8 = fp8(hi) is converted on-device by the (otherwise idle) ACT engine.
  Residual error ~2^-16 relative, far below the top-8 ranking margins.

Device per 512-token block: DMA hi [128, 32*512] fp16 and lo8 fp8 (>=512B
contiguous runs); per 128-token tile: 32 fp16 matmuls (P1) + 32 DR matmuls
(P2), logits combine on DVE, ACT sigmoid, group top-2-sum -> top-4 groups ->
top-8 experts via DVE max8/max_index (first-occurrence ties match
jax.lax.top_k). Outputs idx + top-8 ms-values; the weight gather happens on
host: sig[idx] = v8 - bias[idx] exactly (group penalty is 0 on selected
groups), then normalize * 2.5.
"""
import numpy as np

BSZ, SEQ, H, E = 4, 4096, 4096, 256
N_TOK = BSZ * SEQ
N_CORES = 8
TOK_PER_CORE = N_TOK // N_CORES          # 2048
N_TILES = TOK_PER_CORE // 128            # 16
KT = H // 128                            # 32 k-tiles
G = 8                                    # expert groups
GSZ = E // G                             # 32 experts/group
NB = 4                                   # tiles per DMA block
TB = NB * 128                            # tokens per block
NBLK = N_TILES // NB                     # 4 blocks

_compiled = None


def _build():
    import concourse.bacc as bacc
    import concourse.mybir as mybir
    import concourse.tile as tile

    dt = mybir.dt
    AF = mybir.ActivationFunctionType
    op = mybir.AluOpType

    NB = 4
    TB = NB * 128
    nc = bacc.Bacc("TRN2", target_bir_lowering=False, debug=False,
                   num_devices=N_CORES)
    HI = nc.dram_tensor("hi", [N_TILES // NB, KT, 128, TB], dt.float16,
                        kind="ExternalInput").ap()
    LO8 = nc.dram_tensor("lo8", [N_TILES // NB, KT, 128, TB], dt.float8e4,
                         kind="ExternalInput").ap()
    WHI = nc.dram_tensor("whi", [H, E], dt.float16,
                         kind="ExternalInput").ap()
    W8P = nc.dram_tensor("w8p", [H, E, 2], dt.float8e4,
                         kind="ExternalInput").ap()
    BIAS = nc.dram_tensor("biasrep", [128, E], dt.float32,
                          kind="ExternalInput").ap()
    IDX = nc.dram_tensor("IDX", [TOK_PER_CORE, 8], dt.uint32,
                         kind="ExternalOutput").ap()
    VV = nc.dram_tensor("VV", [TOK_PER_CORE, 8], dt.float32,
                        kind="ExternalOutput").ap()
    LGO = nc.dram_tensor("LGO", [NB, 128, E], dt.float32,
                         kind="ExternalOutput").ap()

    with tile.TileContext(nc) as tc:
        with (
            tc.tile_pool(name="const", bufs=1) as cpool,
            tc.tile_pool(name="hi", bufs=2) as hipool,
            tc.tile_pool(name="cr", bufs=2) as crpool,
            tc.tile_pool(name="work", bufs=4) as wpool,
            tc.tile_pool(name="small", bufs=3) as spool,
            tc.tile_pool(name="psl", bufs=6, space="PSUM") as ppl,
            tc.tile_pool(name="psc", bufs=2, space="PSUM") as ppc,
        ):
            whi_sb = cpool.tile([128, KT * E], dt.float16, tag="whi")
            WHI3 = WHI.rearrange("(k p) e -> p k e", p=128)
            WHIS = whi_sb[:].rearrange("p (k e) -> p k e", k=KT)
            w8_sb = cpool.tile([128, KT * E * 2], dt.float8e4, tag="w8")
            W8P4 = W8P.rearrange("(k p) e s -> p k e s", p=128)
            W8S = w8_sb[:].rearrange("p (k e s) -> p k e s", k=KT, s=2)
            bias_sb = cpool.tile([128, E], dt.float32, tag="bias")

            def load_weights():
                for c in range(4):
                    nc.sync.dma_start(WHIS[:, c * 8:(c + 1) * 8, :],
                                      WHI3[:, c * 8:(c + 1) * 8, :])
                for c in range(4):
                    nc.sync.dma_start(W8S[:, c * 8:(c + 1) * 8],
                                      W8P4[:, c * 8:(c + 1) * 8])
                nc.sync.dma_start(bias_sb[:], BIAS)

            iall = cpool.tile([128, N_TILES * 8], dt.uint32, tag="iall")
            vall = cpool.tile([128, N_TILES * 8], dt.float32, tag="vall")

            hi_sbs = [None] * (N_TILES // NB)
            cr_sbs = [None] * (N_TILES // NB)

            def alloc_block(b):
                hi_t = hipool.tile([128, KT * TB], dt.float16, tag="hi")
                hi_sbs[b] = hi_t
                cr_t = crpool.tile([128, 2 * KT * TB], dt.float8e4, tag="cr")
                cr_sbs[b] = cr_t
                return (hi_t[:].rearrange("p (k j) -> p k j", k=KT),
                        HI[b].rearrange("k p j -> p k j"),
                        cr_t[:, :KT * TB].rearrange("p (k j) -> p k j", k=KT),
                        LO8[b].rearrange("k p j -> p k j"))

            def dma_block(b):
                HIV, HID, CRV, LOD = alloc_block(b)
                th = TB // 2
                nc.sync.dma_start(HIV[:, :, :th], HID[:, :, :th])
                nc.sync.dma_start(HIV[:, :, th:], HID[:, :, th:])
                nc.sync.dma_start(CRV, LOD)

            def dma_startup():
                # blocks 0 and 1 with whi/w8/bias interleaved so the PE can
                # start early and never starves before steady state
                HIV0, HID0, CRV0, LOD0 = alloc_block(0)
                HIV1, HID1, CRV1, LOD1 = alloc_block(1)
                th = TB // 2
                kh = KT // 2
                # quarters ordered to match P1 emission: ph0 (k<16) for
                # tiles 0/1 then 2/3, then ph1 (k>=16) likewise
                nc.sync.dma_start(HIV0[:, :kh, :th], HID0[:, :kh, :th])
                nc.sync.dma_start(WHIS[:, 0:8, :], WHI3[:, 0:8, :])
                nc.sync.dma_start(WHIS[:, 8:16, :], WHI3[:, 8:16, :])
                nc.sync.dma_start(HIV0[:, :kh, th:], HID0[:, :kh, th:])
                nc.sync.dma_start(HIV0[:, kh:, :th], HID0[:, kh:, :th])
                nc.sync.dma_start(WHIS[:, 16:24, :], WHI3[:, 16:24, :])
                nc.sync.dma_start(WHIS[:, 24:32, :], WHI3[:, 24:32, :])
                nc.sync.dma_start(HIV0[:, kh:, th:], HID0[:, kh:, th:])
                nc.sync.dma_start(CRV0, LOD0)
                for c in range(4):
                    nc.sync.dma_start(W8S[:, c * 8:(c + 1) * 8],
                                      W8P4[:, c * 8:(c + 1) * 8])
                nc.sync.dma_start(HIV1[:, :, :th], HID1[:, :, :th])
                nc.sync.dma_start(HIV1[:, :, th:], HID1[:, :, th:])
                nc.sync.dma_start(bias_sb[:], BIAS)
                nc.sync.dma_start(CRV1, LOD1)

            def conv_tile(b, tt):
                hi_t, cr_t = hi_sbs[b], cr_sbs[b]
                HI4 = hi_t[:].rearrange("p (k j) -> p k j", k=KT)
                CRS1 = cr_t[:, KT * TB:].rearrange("p (k j) -> p k j", k=KT)
                nc.scalar.activation(
                    CRS1[:, :, tt * 128:(tt + 1) * 128],
                    HI4[:, :, tt * 128:(tt + 1) * 128], AF.Copy)

            dma_startup()
            for tt in range(NB):
                conv_tile(0, tt)

            plogs = {}
            pcrs = {}

            def do_p1(b, tt, phase=None):
                hi_sb = hi_sbs[b]
                if phase in (None, 0):
                    plog_t = ppl.tile([128, E], dt.float32, tag="lg")
                    plogs[(b, tt)] = plog_t
                else:
                    plog_t = plogs[(b, tt)]
                kr = (range(KT) if phase is None else
                      range(KT // 2) if phase == 0 else
                      range(KT // 2, KT))
                for k in kr:
                    nc.tensor.matmul(
                        plog_t[:],
                        hi_sb[:, k * TB + tt * 128:k * TB + tt * 128 + 128],
                        whi_sb[:, k * E:(k + 1) * E],
                        start=(k == 0), stop=(k == KT - 1))

            def do_dr(b, tt):
                cr_sb = cr_sbs[b]
                CR4 = cr_sb[:].rearrange("p (s k j) -> p k s j", s=2, k=KT)
                pcr_t = ppc.tile([128, E], dt.float32, tag="cr")
                pcrs[(b, tt)] = pcr_t
                for k in range(KT):
                    nc.tensor.matmul(
                        pcr_t[:],
                        CR4[:, k, :, tt * 128:tt * 128 + 128],
                        W8K[:, k],
                        start=(k == 0), stop=(k == KT - 1),
                        perf_mode=mybir.MatmulPerfMode.DoubleRow)

            lgs = {}

            def combine(b, tt):
                pcr = pcrs[(b, tt)]
                plog = plogs[(b, tt)]
                cr_s = wpool.tile([128, E], dt.float32, tag="crs")
                nc.vector.tensor_scalar(cr_s[:], pcr[:],
                                        float(2.0 ** -18), None, op.mult)
                lg = wpool.tile([128, E], dt.float32, tag="lgs")
                nc.vector.tensor_tensor(lg[:], cr_s[:], plog[:], op.add)
                lgs[(b, tt)] = lg

            def routing(b, tt):
                t = b * NB + tt
                lg = lgs[(b, tt)]
                if b == N_TILES // NB - 1:
                    nc.sync.dma_start(LGO[tt], lg[:])
                    return
                sig = wpool.tile([128, E], dt.float32, tag="sig")
                nc.scalar.activation(sig[:], lg[:], AF.Sigmoid)
                if b + 1 < N_TILES // NB:
                    conv_tile(b + 1, tt)
                S = wpool.tile([128, E], dt.float32, tag="S")
                nc.vector.tensor_tensor(S[:], sig[:], bias_sb[:], op.add)
                m1 = spool.tile([128, G], dt.float32, tag="m1")
                S3 = S[:].rearrange("p (g z) -> p g z", g=G)
                nc.vector.tensor_reduce(m1[:], S3, axis=mybir.AxisListType.X,
                                        op=op.max)
                Sm = wpool.tile([128, E], dt.float32, tag="Sm")
                nc.vector.match_replace(Sm[:], m1[:], S[:], -1e30)
                m2 = spool.tile([128, G], dt.float32, tag="m2")
                nc.vector.tensor_reduce(
                    m2[:], Sm[:].rearrange("p (g z) -> p g z", g=G),
                    axis=mybir.AxisListType.X, op=op.max)
                gs = spool.tile([128, G], dt.float32, tag="gs")
                nc.vector.tensor_tensor(gs[:], m1[:], m2[:], op.add)
                gt = spool.tile([128, G * G], dt.float32, tag="gt")
                ga = gs[:].unsqueeze(1).broadcast_to([128, G, G])
                gb_ = gs[:].unsqueeze(2).broadcast_to([128, G, G])
                nc.vector.tensor_tensor(
                    gt[:].rearrange("p (a b) -> p a b", a=G), ga, gb_,
                    op.is_gt)
                cnt = spool.tile([128, G], dt.float32, tag="cnt")
                nc.vector.tensor_reduce(
                    cnt[:], gt[:].rearrange("p (a b) -> p a b", a=G),
                    axis=mybir.AxisListType.X, op=op.add)
                pen = spool.tile([128, G], dt.float32, tag="pen")
                nc.vector.tensor_scalar(pen[:], cnt[:], 3.5, -1e30,
                                        op.is_gt, op.mult)
                ms = wpool.tile([128, E], dt.float32, tag="ms")
                nc.vector.tensor_tensor(
                    ms[:].rearrange("p (g z) -> p g z", g=G),
                    S3,
                    pen[:].unsqueeze(2).broadcast_to([128, G, GSZ]),
                    op.add)
                nc.vector.max(vall[:, t * 8:(t + 1) * 8], ms[:])
                nc.vector.max_index(iall[:, t * 8:(t + 1) * 8],
                                    vall[:, t * 8:(t + 1) * 8], ms[:])

            W8K = w8_sb[:].rearrange("p (k e s) -> p k s e", k=KT, s=2)

            # block 0 + head of block 1: phased P1s, then DRs with combines
            # interleaved (frees PSUM banks: peak 6 plog + 2 pcr = 8 banks)
            do_p1(0, 0, phase=0)
            do_p1(0, 1, phase=0)
            do_p1(0, 2, phase=0)
            do_p1(0, 3, phase=0)
            do_p1(0, 0, phase=1)
            do_p1(0, 1, phase=1)
            do_p1(0, 2, phase=1)
            do_p1(0, 3, phase=1)
            for tt in range(NB):
                do_dr(0, tt)
                combine(0, tt)
            for tt in range(NB):
                routing(0, tt)

            # block 1
            dma_block(2)
            do_p1(1, 0)
            do_p1(1, 1)
            do_p1(1, 2)
            do_p1(1, 3)
            for tt in range(NB):
                do_dr(1, tt)
                combine(1, tt)
            for tt in range(NB):
                routing(1, tt)

            # steady blocks
            for b in (2, 3):
                if b + 1 < N_TILES // NB:
                    dma_block(b + 1)
                last = b == N_TILES // NB - 1
                do_p1(b, 0)
                do_p1(b, 1)
                do_dr(b, 0)
                combine(b, 0)
                if last:
                    routing(b, 0)
                do_p1(b, 2)
                do_dr(b, 1)
                combine(b, 1)
                if last:
                    routing(b, 1)
                do_p1(b, 3)
                do_dr(b, 2)
                combine(b, 2)
                if last:
                    routing(b, 2)
                do_dr(b, 3)
                combine(b, 3)
                if last:
                    routing(b, 3)
                else:
                    for tt in range(NB):
                        routing(b, tt)

            IDX4 = IDX.rearrange("(t p) j -> p t j", p=128)
            VV4 = VV.rearrange("(t p) j -> p t j", p=128)
            IALL4 = iall[:].rearrange("p (t j) -> p t j", t=N_TILES)
            VALL4 = vall[:].rearrange("p (t j) -> p t j", t=N_TILES)
            nt0 = N_TILES - NB
            nc.sync.dma_start(IDX4[:, :nt0], IALL4[:, :nt0])
            nc.sync.dma_start(VV4[:, :nt0], VALL4[:, :nt0])

    nc.compile()
    return nc


def kernel(hidden_states, weight, e_score_correction_bias):
    global _compiled
    import ml_dtypes
    from concourse import bass_utils

    f8 = ml_dtypes.float8_e4m3fn

    hs = np.asarray(hidden_states, dtype=np.float32).reshape(N_TOK, H)
    w = np.asarray(weight, dtype=np.float32)
    bias = np.asarray(e_score_correction_bias, dtype=np.float32)
    biasrep = np.ascontiguousarray(np.tile(bias[None, :], (128, 1)))

    # host-side splits for the w side (shared across cores)
    wt = np.ascontiguousarray(w.T)                        # [H, E] fp32
    whi = wt.astype(np.float16)                           # [H, E]
    wlo = wt - whi.astype(np.float32)
    w8p = np.empty((H, E, 2), dtype=f8)
    w8p[:, :, 0] = (wt * np.float32(2.0 ** 6)).astype(f8)
    w8p[:, :, 1] = (wlo * np.float32(2.0 ** 18)).astype(f8)
    w8p = np.ascontiguousarray(w8p)

    if _compiled is None:
        _compiled = _build()
    nc = _compiled

    in_maps = []
    for c in range(N_CORES):
        sl = hs[c * TOK_PER_CORE:(c + 1) * TOK_PER_CORE]  # [2048, 4096]
        # [H, TOK] transposed, then to block layout [NBLK, KT, 128, TB]
        hst = np.ascontiguousarray(sl.T)                  # [4096, 2048]
        hi32 = hst.astype(np.float16).astype(np.float32)
        lo = hst - hi32
        hi_b = np.ascontiguousarray(
            hi32.astype(np.float16).reshape(KT, 128, NBLK, TB)
            .transpose(2, 0, 1, 3))
        lo8_b = np.ascontiguousarray(
            (lo * np.float32(2.0 ** 12)).astype(f8).reshape(KT, 128, NBLK, TB)
            .transpose(2, 0, 1, 3))
        in_maps.append({"hi": hi_b, "lo8": lo8_b, "whi": whi, "w8p": w8p,
                        "biasrep": biasrep})

    res = bass_utils.run_bass_kernel_spmd(
        nc, in_maps=in_maps, core_ids=list(range(N_CORES)))

    idx = np.concatenate([res.results[c]["IDX"] for c in range(N_CORES)],
                         axis=0).astype(np.int64)
    v8 = np.concatenate([res.results[c]["VV"] for c in range(N_CORES)],
                        axis=0).astype(np.float32)
    # host epilogue: sig[idx] = v8 - bias[idx] (pen==0 on selected groups)
    sig8 = v8 - bias[idx]

    # last block of each core (4 tiles): full routing on host from logits
    for c in range(N_CORES):
        lgo = np.asarray(res.results[c]["LGO"], dtype=np.float32)
        lgo = lgo.reshape(NB * 128, E)   # token = tt*128 + p
        scores = (1.0 / (1.0 + np.exp(-lgo.astype(np.float64)))).astype(
            np.float32)
        sfc = scores + bias[None, :]
        n = NB * 128
        grp = sfc.reshape(n, G, GSZ)
        top2 = np.sort(grp, axis=-1)[:, :, -2:]
        gsc = top2.sum(axis=-1)
        gidx = np.argsort(-gsc, axis=1, kind="stable")[:, :4]
        gmask = np.zeros((n, G), dtype=bool)
        np.put_along_axis(gmask, gidx, True, axis=1)
        smask = np.repeat(gmask, GSZ, axis=1)
        tmp = np.where(smask, sfc, -np.inf)
        ti = np.argsort(-tmp, axis=1, kind="stable")[:, :8]
        rows = slice(c * TOK_PER_CORE + TOK_PER_CORE - n,
                     (c + 1) * TOK_PER_CORE)
        idx[rows] = ti
        sig8[rows] = np.take_along_axis(scores, ti, axis=1)

    denom = sig8.sum(axis=-1, keepdims=True, dtype=np.float32) + np.float32(
        1e-20)
    wout = (sig8 / denom) * np.float32(2.5)
    return idx.astype(np.int32), wout.astype(np.float32)



# revision 2
# speedup vs baseline: 1.0715x; 1.0715x over previous
"""MoE gate routing kernel for Trainium2 (8 NeuronCores, SPMD token-parallel).

v6: fp32r-P1 + single-fp8-DR-P2, DMA-paced schedule.

Problem: hidden_states [4,4096,4096] f32, weight [256,4096] f32, bias [256] f32.
reference: logits = hs @ W.T; scores = sigmoid(logits); grouped top-2-sum group
scores -> top-4 groups -> top-8 experts; returns (topk_idx int32 [n,8],
topk_weight f32 [n,8]) with weights = normalized sigmoid scores * 2.5.

Sharding: token dim (n = 16384) split across 8 cores (2048 tokens each); the
gate weight and bias are replicated.

GEMM decomposition (splits on host):
  x = x16 (fp16) + xlo,   xlo8 = fp8(xlo * 2^12)
  logits = x16 @ w        (P1: fp32r matmuls -- w at full fp32 precision,
                           x16 converted fp16->fp32r on ACT(h0)+Pool(h1))
         + 2^-18 * xlo8 @ w8    (P2: one fp8 DoubleRow pass; adjacent
                           k-blocks pair via strided views; w8 = fp8(w*2^6)
                           derived on-device from w by the DVE)
  Error ~1.3e-4 logit std (fp32r internal rounding) + ~2^-15 split residue;
  measured end-to-end max(rel_idx, rel_w) ~ 1.56e-2 (43/16384 near-tie rows).

Schedule: the whole 28 MiB input stream (x16+xlo8 p-major tiles, w fp32r)
is emitted up front in arrival order; deep SBUF rings keep the DMA engines
saturated; the PE trails the stream by one conversion latency. The final
tile's x16 ships in halves and converts/multiplies in quarters to shrink
the tail. The last 4 tiles' logits ship to host (LGO) and route there.
Outputs idx + top-8 ms-values; weight gather on host: sig[idx] = v8 -
bias[idx] (group penalty is 0 on selected groups), then normalize * 2.5.
"""
import numpy as np

BSZ, SEQ, H, E = 4, 4096, 4096, 256
N_TOK = BSZ * SEQ
N_CORES = 8
TOK_PER_CORE = N_TOK // N_CORES          # 2048
N_TILES = TOK_PER_CORE // 128            # 16
KT = H // 128                            # 32 k-tiles
G = 8                                    # expert groups
GSZ = E // G                             # 32 experts/group
KP = KT // 2                             # 16 k-block pairs (DR)
NLGO = 4                                 # last tiles routed on host
TW = KT * 128                            # x elems per tile per partition

_compiled = None


def _build():
    import concourse.bacc as bacc
    import concourse.mybir as mybir
    import concourse.tile as tile

    dt = mybir.dt
    AF = mybir.ActivationFunctionType
    op = mybir.AluOpType

    nc = bacc.Bacc("TRN2", target_bir_lowering=False, debug=False,
                   num_devices=N_CORES)
    # p-major: [tile, partition, k*j] -- 8/16KB contiguous per partition
    X16 = nc.dram_tensor("x16", [N_TILES, 128, TW], dt.float16,
                         kind="ExternalInput").ap()
    XL8 = nc.dram_tensor("xl8", [N_TILES, 128, TW], dt.float8e4,
                         kind="ExternalInput").ap()
    WR = nc.dram_tensor("wr", [H, E], dt.float32r,
                        kind="ExternalInput").ap()
    BIAS = nc.dram_tensor("biasrep", [128, E], dt.float32,
                          kind="ExternalInput").ap()
    IDX = nc.dram_tensor("IDX", [TOK_PER_CORE, 8], dt.uint32,
                         kind="ExternalOutput").ap()
    VV = nc.dram_tensor("VV", [TOK_PER_CORE, 8], dt.float32,
                        kind="ExternalOutput").ap()
    LGO = nc.dram_tensor("LGO", [NLGO, 128, E], dt.float32,
                         kind="ExternalOutput").ap()

    with tile.TileContext(nc) as tc:
        with (
            tc.tile_pool(name="const", bufs=1) as cpool,
            tc.tile_pool(name="hi", bufs=6) as hipool,
            tc.tile_pool(name="cr", bufs=8) as crpool,
            tc.tile_pool(name="xr", bufs=4) as xrpool,
            tc.tile_pool(name="work", bufs=4) as wpool,
            tc.tile_pool(name="small", bufs=3) as spool,
            tc.tile_pool(name="psl", bufs=4, space="PSUM") as ppl,
            tc.tile_pool(name="psc", bufs=3, space="PSUM") as ppc,
        ):
            w_sb = cpool.tile([128, KT * E], dt.float32r, tag="wr")
            WR3 = WR.rearrange("(k p) e -> p k e", p=128)
            WRS = w_sb[:].rearrange("p (k e) -> p k e", k=KT)
            w8_sb = cpool.tile([128, KT * E], dt.float8e4, tag="w8")
            W8F = w8_sb[:].rearrange("p (k e) -> p k e", k=KT)
            W8S = w8_sb[:].rearrange("p (k s e) -> p k s e", k=KP, s=2)
            bias_sb = cpool.tile([128, E], dt.float32, tag="bias")

            iall = cpool.tile([128, N_TILES * 8], dt.uint32, tag="iall")
            vall = cpool.tile([128, N_TILES * 8], dt.float32, tag="vall")

            hi_sbs = {}
            cr_sbs = {}
            xr_sbs = {}

            def alloc_tile(t):
                hi_t = hipool.tile([128, TW], dt.float16, tag="hi")
                hi_sbs[t] = hi_t
                cr_t = crpool.tile([128, TW], dt.float8e4, tag="cr")
                cr_sbs[t] = cr_t
                return hi_t, cr_t

            def conv_tile(t):
                # h0 on ACT, h1 on Pool -- halves convert in parallel
                hi_t = hi_sbs[t]
                xr_t = xrpool.tile([128, TW], dt.float32r, tag="xr")
                xr_sbs[t] = xr_t
                h = TW // 2
                nc.scalar.activation(xr_t[:, :h], hi_t[:, :h], AF.Copy)
                nc.gpsimd.tensor_copy(xr_t[:, h:], hi_t[:, h:])

            plogs = {}
            pcrs = {}

            def do_p1(t, q=None):
                xr_t = xr_sbs[t]
                XR3 = xr_t[:].rearrange("p (k j) -> p k j", k=KT)
                if q in (None, 0):
                    plog_t = ppl.tile([128, E], dt.float32, tag="lg")
                    plogs[t] = plog_t
                else:
                    plog_t = plogs[t]
                kr = range(KT) if q is None else range(8 * q, 8 * (q + 1))  # q: 8-k chunks
                for k in kr:
                    nc.tensor.matmul(
                        plog_t[:], XR3[:, k], WRS[:, k],
                        start=(k == 0), stop=(k == KT - 1))

            def do_dr(t, c=None):
                cr_t = cr_sbs[t]
                CR4 = cr_t[:].rearrange("p (k s j) -> p k s j", k=KP, s=2)
                if c in (None, 0):
                    pcr_t = ppc.tile([128, E], dt.float32, tag="cr")
                    pcrs[t] = pcr_t
                else:
                    pcr_t = pcrs[t]
                kr = range(KP) if c is None else range(4 * c, 4 * (c + 1))
                for k in kr:
                    nc.tensor.matmul(
                        pcr_t[:], CR4[:, k], W8S[:, k],
                        start=(k == 0), stop=(k == KP - 1),
                        perf_mode=mybir.MatmulPerfMode.DoubleRow)

            lgs = {}

            def combine(t):
                pcr = pcrs.pop(t)
                plog = plogs.pop(t)
                cr_s = wpool.tile([128, E], dt.float32, tag="crs")
                nc.vector.tensor_scalar(cr_s[:], pcr[:],
                                        float(2.0 ** -18), None, op.mult)
                lg = wpool.tile([128, E], dt.float32, tag="lgs")
                nc.vector.tensor_tensor(lg[:], cr_s[:], plog[:], op.add)
                lgs[t] = lg

            def routing(t):
                lg = lgs.pop(t)
                if t >= N_TILES - NLGO:
                    nc.sync.dma_start(LGO[t - (N_TILES - NLGO)], lg[:])
                    return
                sig = wpool.tile([128, E], dt.float32, tag="sig")
                nc.scalar.activation(sig[:], lg[:], AF.Sigmoid)
                S = wpool.tile([128, E], dt.float32, tag="S")
                nc.vector.tensor_tensor(S[:], sig[:], bias_sb[:], op.add)
                m1 = spool.tile([128, G], dt.float32, tag="m1")
                S3 = S[:].rearrange("p (g z) -> p g z", g=G)
                nc.vector.tensor_reduce(m1[:], S3, axis=mybir.AxisListType.X,
                                        op=op.max)
                Sm = wpool.tile([128, E], dt.float32, tag="Sm")
                nc.vector.match_replace(Sm[:], m1[:], S[:], -1e30)
                m2 = spool.tile([128, G], dt.float32, tag="m2")
                nc.vector.tensor_reduce(
                    m2[:], Sm[:].rearrange("p (g z) -> p g z", g=G),
                    axis=mybir.AxisListType.X, op=op.max)
                gs = spool.tile([128, G], dt.float32, tag="gs")
                nc.vector.tensor_tensor(gs[:], m1[:], m2[:], op.add)
                gt = spool.tile([128, G * G], dt.float32, tag="gt")
                ga = gs[:].unsqueeze(1).broadcast_to([128, G, G])
                gb_ = gs[:].unsqueeze(2).broadcast_to([128, G, G])
                nc.vector.tensor_tensor(
                    gt[:].rearrange("p (a b) -> p a b", a=G), ga, gb_,
                    op.is_gt)
                cnt = spool.tile([128, G], dt.float32, tag="cnt")
                nc.vector.tensor_reduce(
                    cnt[:], gt[:].rearrange("p (a b) -> p a b", a=G),
                    axis=mybir.AxisListType.X, op=op.add)
                pen = spool.tile([128, G], dt.float32, tag="pen")
                nc.vector.tensor_scalar(pen[:], cnt[:], 3.5, -1e30,
                                        op.is_gt, op.mult)
                # ms reuses Sm's buffer (Sm is dead after m2)
                ms = wpool.tile([128, E], dt.float32, tag="Sm")
                nc.vector.tensor_tensor(
                    ms[:].rearrange("p (g z) -> p g z", g=G),
                    S3,
                    pen[:].unsqueeze(2).broadcast_to([128, G, GSZ]),
                    op.add)
                nc.vector.max(vall[:, t * 8:(t + 1) * 8], ms[:])
                nc.vector.max_index(iall[:, t * 8:(t + 1) * 8],
                                    vall[:, t * 8:(t + 1) * 8], ms[:])

            # ---- DMA stream, emitted up front in arrival order ----
            h = TW // 2
            for t in range(N_TILES):
                alloc_tile(t)
            hi15 = hi_sbs[N_TILES - 1]
            nc.sync.dma_start(hi_sbs[0][:], X16[0])
            nc.sync.dma_start(WRS[:, 0:8, :], WR3[:, 0:8, :])
            nc.sync.dma_start(WRS[:, 8:16, :], WR3[:, 8:16, :])
            nc.sync.dma_start(hi_sbs[1][:], X16[1])
            nc.sync.dma_start(WRS[:, 16:24, :], WR3[:, 16:24, :])
            nc.sync.dma_start(WRS[:, 24:32, :], WR3[:, 24:32, :])
            nc.sync.dma_start(bias_sb[:], BIAS)
            nc.sync.dma_start(hi_sbs[2][:], X16[2])
            for t in range(3, N_TILES - 1):
                nc.sync.dma_start(hi_sbs[t][:], X16[t])
                nc.sync.dma_start(cr_sbs[t - 3][:], XL8[t - 3])
            nc.sync.dma_start(cr_sbs[N_TILES - 4][:], XL8[N_TILES - 4])
            nc.sync.dma_start(hi15[:, :h], X16[N_TILES - 1][:, :h])
            nc.sync.dma_start(hi15[:, h:], X16[N_TILES - 1][:, h:])
            nc.sync.dma_start(cr_sbs[N_TILES - 3][:], XL8[N_TILES - 3])
            nc.sync.dma_start(cr_sbs[N_TILES - 2][:], XL8[N_TILES - 2])
            nc.sync.dma_start(cr_sbs[N_TILES - 1][:], XL8[N_TILES - 1])

            # ---- derive w8 = fp8(w * 2^6) on the DVE (idle early) ----
            for c in range(4):
                nc.vector.tensor_scalar(
                    W8F[:, c * 8:(c + 1) * 8], WRS[:, c * 8:(c + 1) * 8],
                    float(2.0 ** 6), None, op.mult)

            conv_tile(0)
            conv_tile(1)

            # ---- PE stream: P1 one tile ahead, conv two ahead ----
            do_p1(0)
            do_p1(1)
            TL = N_TILES - 1
            for t in range(N_TILES):
                if t + 2 <= TL - 1:
                    conv_tile(t + 2)
                    if t < TL:
                        do_dr(t, c=0)
                    do_p1(t + 2, q=0)
                    if t < TL:
                        do_dr(t, c=1)
                    do_p1(t + 2, q=1)
                    if t < TL:
                        do_dr(t, c=2)
                    do_p1(t + 2, q=2)
                    if t < TL:
                        do_dr(t, c=3)
                    do_p1(t + 2, q=3)
                elif t < TL:
                    do_dr(t)
                if t == TL:
                    # last tile: DR first (xlo8 arrives before x16),
                    # then P1 in quarters as conversion quarters land
                    do_dr(TL)
                    xr_t = xrpool.tile([128, TW], dt.float32r, tag="xr")
                    xr_sbs[TL] = xr_t
                    qq = TW // 4
                    nc.scalar.activation(xr_t[:, 0:qq], hi15[:, 0:qq],
                                         AF.Copy)
                    do_p1(TL, q=0)
                    nc.scalar.activation(xr_t[:, qq:2 * qq],
                                         hi15[:, qq:2 * qq], AF.Copy)
                    do_p1(TL, q=1)
                    nc.scalar.activation(xr_t[:, 2 * qq:3 * qq],
                                         hi15[:, 2 * qq:3 * qq], AF.Copy)
                    do_p1(TL, q=2)
                    nc.scalar.activation(xr_t[:, 3 * qq:], hi15[:, 3 * qq:],
                                         AF.Copy)
                    do_p1(TL, q=3)
                combine(t)
                if t >= 1:
                    routing(t - 1)
                if t == 8:
                    IDX4 = IDX.rearrange("(t p) j -> p t j", p=128)
                    VV4 = VV.rearrange("(t p) j -> p t j", p=128)
                    IALL4 = iall[:].rearrange("p (t j) -> p t j", t=N_TILES)
                    VALL4 = vall[:].rearrange("p (t j) -> p t j", t=N_TILES)
                    nc.sync.dma_start(IDX4[:, :8], IALL4[:, :8])
                    nc.sync.dma_start(VV4[:, :8], VALL4[:, :8])
                if t == 12:
                    nt0 = N_TILES - NLGO
                    nc.sync.dma_start(IDX4[:, 8:nt0], IALL4[:, 8:nt0])
                    nc.sync.dma_start(VV4[:, 8:nt0], VALL4[:, 8:nt0])
            routing(N_TILES - 1)

    nc.compile()
    return nc


def kernel(hidden_states, weight, e_score_correction_bias):
    global _compiled
    import ml_dtypes
    from concourse import bass_utils

    f8 = ml_dtypes.float8_e4m3fn

    hs = np.asarray(hidden_states, dtype=np.float32).reshape(N_TOK, H)
    w = np.asarray(weight, dtype=np.float32)
    bias = np.asarray(e_score_correction_bias, dtype=np.float32)
    biasrep = np.ascontiguousarray(np.tile(bias[None, :], (128, 1)))

    wt = np.ascontiguousarray(w.T)                        # [H, E] fp32

    if _compiled is None:
        _compiled = _build()
    nc = _compiled

    in_maps = []
    for c in range(N_CORES):
        sl = hs[c * TOK_PER_CORE:(c + 1) * TOK_PER_CORE]  # [2048, 4096]
        hst = np.ascontiguousarray(sl.T)                  # [4096, 2048]
        x16 = hst.astype(np.float16)
        xlo = hst - x16.astype(np.float32)
        xl8 = (xlo * np.float32(2.0 ** 12)).astype(f8)
        # [KT,128,N_TILES,128] -> [tile, p, k, j] p-major
        x16_b = np.ascontiguousarray(
            x16.reshape(KT, 128, N_TILES, 128).transpose(2, 1, 0, 3)
            .reshape(N_TILES, 128, TW))
        xl8_b = np.ascontiguousarray(
            xl8.reshape(KT, 128, N_TILES, 128).transpose(2, 1, 0, 3)
            .reshape(N_TILES, 128, TW))
        in_maps.append({"x16": x16_b, "xl8": xl8_b, "wr": wt,
                        "biasrep": biasrep})

    res = bass_utils.run_bass_kernel_spmd(
        nc, in_maps=in_maps, core_ids=list(range(N_CORES)))

    idx = np.concatenate([res.results[c]["IDX"] for c in range(N_CORES)],
                         axis=0).astype(np.int64)
    v8 = np.concatenate([res.results[c]["VV"] for c in range(N_CORES)],
                        axis=0).astype(np.float32)
    # host epilogue: sig[idx] = v8 - bias[idx] (pen==0 on selected groups)
    sig8 = v8 - bias[idx]

    # last NLGO tiles of each core: full routing on host from logits
    for c in range(N_CORES):
        lgo = np.asarray(res.results[c]["LGO"], dtype=np.float32)
        lgo = lgo.reshape(NLGO * 128, E)   # token = slot*128 + p
        scores = (1.0 / (1.0 + np.exp(-lgo.astype(np.float64)))).astype(
            np.float32)
        sfc = scores + bias[None, :]
        n = NLGO * 128
        grp = sfc.reshape(n, G, GSZ)
        top2 = np.sort(grp, axis=-1)[:, :, -2:]
        gsc = top2.sum(axis=-1)
        gidx = np.argsort(-gsc, axis=1, kind="stable")[:, :4]
        gmask = np.zeros((n, G), dtype=bool)
        np.put_along_axis(gmask, gidx, True, axis=1)
        smask = np.repeat(gmask, GSZ, axis=1)
        tmp = np.where(smask, sfc, -np.inf)
        ti = np.argsort(-tmp, axis=1, kind="stable")[:, :8]
        rows = slice(c * TOK_PER_CORE + TOK_PER_CORE - n,
                     (c + 1) * TOK_PER_CORE)
        idx[rows] = ti
        sig8[rows] = np.take_along_axis(scores, ti, axis=1)

    denom = sig8.sum(axis=-1, keepdims=True, dtype=np.float32) + np.float32(
        1e-20)
    wout = (sig8 / denom) * np.float32(2.5)
    return idx.astype(np.int32), wout.astype(np.float32)


# revision 3
# speedup vs baseline: 1.1380x; 1.0620x over previous
"""MoE gate routing kernel for Trainium2 (8 NeuronCores, SPMD token-parallel).

v6: fp32r-P1 + single-fp8-DR-P2, DMA-paced schedule.

Problem: hidden_states [4,4096,4096] f32, weight [256,4096] f32, bias [256] f32.
reference: logits = hs @ W.T; scores = sigmoid(logits); grouped top-2-sum group
scores -> top-4 groups -> top-8 experts; returns (topk_idx int32 [n,8],
topk_weight f32 [n,8]) with weights = normalized sigmoid scores * 2.5.

Sharding: token dim (n = 16384) split across 8 cores (2048 tokens each); the
gate weight and bias are replicated.

GEMM decomposition (splits on host):
  x = x16 (fp16) + xlo,   xlo8 = fp8(xlo * 2^12)
  logits = x16 @ w        (P1: fp32r matmuls -- w at full fp32 precision,
                           x16 converted fp16->fp32r on ACT(h0)+Pool(h1))
         + 2^-18 * xlo8 @ w8    (P2: one fp8 DoubleRow pass; adjacent
                           k-blocks pair via strided views; w8 = fp8(w*2^6)
                           derived on-device from w by the DVE)
  Error ~1.3e-4 logit std (fp32r internal rounding) + ~2^-15 split residue;
  measured end-to-end max(rel_idx, rel_w) ~ 1.56e-2 (43/16384 near-tie rows).

Schedule: the whole 28 MiB input stream (x16+xlo8 p-major tiles, w fp32r)
is emitted up front in arrival order; deep SBUF rings keep the DMA engines
saturated; the PE trails the stream by one conversion latency. The final
tile's x16 ships in halves and converts/multiplies in quarters to shrink
the tail. The last 4 tiles' logits ship to host (LGO) and route there.
Outputs idx + top-8 ms-values; weight gather on host: sig[idx] = v8 -
bias[idx] (group penalty is 0 on selected groups), then normalize * 2.5.
"""
import numpy as np

BSZ, SEQ, H, E = 4, 4096, 4096, 256
N_TOK = BSZ * SEQ
N_CORES = 8
TOK_PER_CORE = N_TOK // N_CORES          # 2048
N_TILES = TOK_PER_CORE // 128            # 16
KT = H // 128                            # 32 k-tiles
G = 8                                    # expert groups
GSZ = E // G                             # 32 experts/group
KP = KT // 2                             # 16 k-block pairs (DR)
NLGO = 4                                 # last tiles routed on host
TW = KT * 128                            # x elems per tile per partition

_compiled = None


def _build():
    import concourse.bacc as bacc
    import concourse.mybir as mybir
    import concourse.tile as tile

    dt = mybir.dt
    AF = mybir.ActivationFunctionType
    op = mybir.AluOpType

    nc = bacc.Bacc("TRN2", target_bir_lowering=False, debug=False,
                   num_devices=N_CORES)
    # p-major: [tile, partition, k*j] -- 8/16KB contiguous per partition
    X16 = nc.dram_tensor("x16", [N_TILES, 128, TW], dt.float16,
                         kind="ExternalInput").ap()
    XL8 = nc.dram_tensor("xl8", [N_TILES, 128, TW], dt.float8e4,
                         kind="ExternalInput").ap()
    WR = nc.dram_tensor("wr", [H, E], dt.float32r,
                        kind="ExternalInput").ap()
    BIAS = nc.dram_tensor("biasrep", [128, E], dt.float32,
                          kind="ExternalInput").ap()
    IDX = nc.dram_tensor("IDX", [TOK_PER_CORE, 8], dt.uint32,
                         kind="ExternalOutput").ap()
    VV = nc.dram_tensor("VV", [TOK_PER_CORE, 8], dt.float32,
                        kind="ExternalOutput").ap()
    LGO = nc.dram_tensor("LGO", [NLGO, 128, E], dt.float32,
                         kind="ExternalOutput").ap()

    with tile.TileContext(nc) as tc:
        with (
            tc.tile_pool(name="const", bufs=1) as cpool,
            tc.tile_pool(name="hi", bufs=5) as hipool,
            tc.tile_pool(name="cr", bufs=6) as crpool,
            tc.tile_pool(name="xr", bufs=5) as xrpool,
            tc.tile_pool(name="work", bufs=4) as wpool,
            tc.tile_pool(name="small", bufs=3) as spool,
            tc.tile_pool(name="psl", bufs=4, space="PSUM") as ppl,
            tc.tile_pool(name="psc", bufs=3, space="PSUM") as ppc,
        ):
            w_sb = cpool.tile([128, KT * E], dt.float32r, tag="wr")
            WR3 = WR.rearrange("(k p) e -> p k e", p=128)
            WRS = w_sb[:].rearrange("p (k e) -> p k e", k=KT)
            w8_sb = cpool.tile([128, KT * E], dt.float8e4, tag="w8")
            W8F = w8_sb[:].rearrange("p (k e) -> p k e", k=KT)
            W8S = w8_sb[:].rearrange("p (k s e) -> p k s e", k=KP, s=2)
            bias_sb = cpool.tile([128, E], dt.float32, tag="bias")

            iall = cpool.tile([128, N_TILES * 8], dt.uint32, tag="iall")
            vall = cpool.tile([128, N_TILES * 8], dt.float32, tag="vall")

            hi_sbs = {}
            cr_sbs = {}
            xr_sbs = {}

            def alloc_tile(t):
                hi_t = hipool.tile([128, TW], dt.float16, tag="hi")
                hi_sbs[t] = hi_t
                cr_t = crpool.tile([128, TW], dt.float8e4, tag="cr")
                cr_sbs[t] = cr_t
                return hi_t, cr_t

            def conv_tile(t):
                # h0 on ACT, h1 on Pool -- halves convert in parallel
                hi_t = hi_sbs[t]
                xr_t = xrpool.tile([128, TW], dt.float32r, tag="xr")
                xr_sbs[t] = xr_t
                h = TW // 2
                nc.scalar.activation(xr_t[:, :h], hi_t[:, :h], AF.Copy)
                nc.gpsimd.tensor_copy(xr_t[:, h:], hi_t[:, h:])

            plogs = {}
            pcrs = {}

            def do_p1(t, q=None):
                xr_t = xr_sbs[t]
                XR3 = xr_t[:].rearrange("p (k j) -> p k j", k=KT)
                if q in (None, 0):
                    plog_t = ppl.tile([128, E], dt.float32, tag="lg")
                    plogs[t] = plog_t
                else:
                    plog_t = plogs[t]
                kr = range(KT) if q is None else range(8 * q, 8 * (q + 1))  # q: 8-k chunks
                for k in kr:
                    nc.tensor.matmul(
                        plog_t[:], XR3[:, k], WRS[:, k],
                        start=(k == 0), stop=(k == KT - 1))

            def do_dr(t, c=None):
                cr_t = cr_sbs[t]
                CR4 = cr_t[:].rearrange("p (k s j) -> p k s j", k=KP, s=2)
                if c in (None, 0):
                    pcr_t = ppc.tile([128, E], dt.float32, tag="cr")
                    pcrs[t] = pcr_t
                else:
                    pcr_t = pcrs[t]
                kr = range(KP) if c is None else range(4 * c, 4 * (c + 1))
                for k in kr:
                    nc.tensor.matmul(
                        pcr_t[:], CR4[:, k], W8S[:, k],
                        start=(k == 0), stop=(k == KP - 1),
                        perf_mode=mybir.MatmulPerfMode.DoubleRow)

            lgs = {}

            def combine(t):
                pcr = pcrs.pop(t)
                plog = plogs.pop(t)
                cr_s = wpool.tile([128, E], dt.float32, tag="crs")
                nc.vector.tensor_scalar(cr_s[:], pcr[:],
                                        float(2.0 ** -18), None, op.mult)
                lg = wpool.tile([128, E], dt.float32, tag="lgs")
                nc.vector.tensor_tensor(lg[:], cr_s[:], plog[:], op.add)
                lgs[t] = lg

            def routing(t):
                lg = lgs.pop(t)
                if t >= N_TILES - NLGO:
                    nc.sync.dma_start(LGO[t - (N_TILES - NLGO)], lg[:])
                    return
                sig = wpool.tile([128, E], dt.float32, tag="sig")
                nc.scalar.activation(sig[:], lg[:], AF.Sigmoid)
                S = wpool.tile([128, E], dt.float32, tag="S")
                nc.vector.tensor_tensor(S[:], sig[:], bias_sb[:], op.add)
                m1 = spool.tile([128, G], dt.float32, tag="m1")
                S3 = S[:].rearrange("p (g z) -> p g z", g=G)
                nc.vector.tensor_reduce(m1[:], S3, axis=mybir.AxisListType.X,
                                        op=op.max)
                Sm = wpool.tile([128, E], dt.float32, tag="Sm")
                nc.vector.match_replace(Sm[:], m1[:], S[:], -1e30)
                m2 = spool.tile([128, G], dt.float32, tag="m2")
                nc.vector.tensor_reduce(
                    m2[:], Sm[:].rearrange("p (g z) -> p g z", g=G),
                    axis=mybir.AxisListType.X, op=op.max)
                gs = spool.tile([128, G], dt.float32, tag="gs")
                nc.vector.tensor_tensor(gs[:], m1[:], m2[:], op.add)
                gt = spool.tile([128, G * G], dt.float32, tag="gt")
                ga = gs[:].unsqueeze(1).broadcast_to([128, G, G])
                gb_ = gs[:].unsqueeze(2).broadcast_to([128, G, G])
                nc.vector.tensor_tensor(
                    gt[:].rearrange("p (a b) -> p a b", a=G), ga, gb_,
                    op.is_gt)
                cnt = spool.tile([128, G], dt.float32, tag="cnt")
                nc.vector.tensor_reduce(
                    cnt[:], gt[:].rearrange("p (a b) -> p a b", a=G),
                    axis=mybir.AxisListType.X, op=op.add)
                pen = spool.tile([128, G], dt.float32, tag="pen")
                nc.vector.tensor_scalar(pen[:], cnt[:], 3.5, -1e30,
                                        op.is_gt, op.mult)
                # ms reuses Sm's buffer (Sm is dead after m2)
                ms = wpool.tile([128, E], dt.float32, tag="Sm")
                nc.vector.tensor_tensor(
                    ms[:].rearrange("p (g z) -> p g z", g=G),
                    S3,
                    pen[:].unsqueeze(2).broadcast_to([128, G, GSZ]),
                    op.add)
                nc.vector.max(vall[:, t * 8:(t + 1) * 8], ms[:])
                nc.vector.max_index(iall[:, t * 8:(t + 1) * 8],
                                    vall[:, t * 8:(t + 1) * 8], ms[:])

            # ---- DMA stream, emitted up front in arrival order ----
            h = TW // 2
            for t in range(N_TILES):
                alloc_tile(t)
            hi15 = hi_sbs[N_TILES - 1]
            nc.sync.dma_start(hi_sbs[0][:], X16[0])
            nc.sync.dma_start(WRS[:, 0:8, :], WR3[:, 0:8, :])
            nc.sync.dma_start(WRS[:, 8:16, :], WR3[:, 8:16, :])
            nc.sync.dma_start(hi_sbs[1][:], X16[1])
            nc.sync.dma_start(hi_sbs[2][:], X16[2])
            nc.sync.dma_start(WRS[:, 16:24, :], WR3[:, 16:24, :])
            nc.sync.dma_start(WRS[:, 24:32, :], WR3[:, 24:32, :])
            nc.sync.dma_start(bias_sb[:], BIAS)
            q4 = TW // 4
            for t in range(3, N_TILES - 1):
                for qi in range(4):
                    nc.sync.dma_start(hi_sbs[t][:, qi * q4:(qi + 1) * q4],
                                      X16[t][:, qi * q4:(qi + 1) * q4])
                nc.sync.dma_start(cr_sbs[t - 3][:, :h], XL8[t - 3][:, :h])
                nc.sync.dma_start(cr_sbs[t - 3][:, h:], XL8[t - 3][:, h:])
            nc.sync.dma_start(cr_sbs[N_TILES - 4][:], XL8[N_TILES - 4])
            nc.sync.dma_start(hi15[:, :h], X16[N_TILES - 1][:, :h])
            nc.sync.dma_start(hi15[:, h:], X16[N_TILES - 1][:, h:])
            nc.sync.dma_start(cr_sbs[N_TILES - 3][:], XL8[N_TILES - 3])
            nc.sync.dma_start(cr_sbs[N_TILES - 2][:], XL8[N_TILES - 2])
            nc.sync.dma_start(cr_sbs[N_TILES - 1][:], XL8[N_TILES - 1])

            # ---- derive w8 = fp8(w * 2^6) on the DVE (idle early) ----
            for c in range(4):
                nc.vector.tensor_scalar(
                    W8F[:, c * 8:(c + 1) * 8], WRS[:, c * 8:(c + 1) * 8],
                    float(2.0 ** 6), None, op.mult)

            conv_tile(0)
            conv_tile(1)

            # ---- PE stream: P1 one tile ahead, conv two ahead ----
            do_p1(0, q=0)
            do_p1(0, q=1)
            do_p1(1, q=0)
            do_p1(1, q=1)
            do_p1(0, q=2)
            do_p1(0, q=3)
            do_p1(1, q=2)
            do_p1(1, q=3)
            TL = N_TILES - 1
            for t in range(N_TILES):
                if t + 2 <= TL - 1:
                    conv_tile(t + 2)
                    if t < TL:
                        do_dr(t, c=0)
                    do_p1(t + 2, q=0)
                    if t < TL:
                        do_dr(t, c=1)
                    do_p1(t + 2, q=1)
                    if t < TL:
                        do_dr(t, c=2)
                    do_p1(t + 2, q=2)
                    if t < TL:
                        do_dr(t, c=3)
                    do_p1(t + 2, q=3)
                elif t < TL:
                    do_dr(t)
                if t == TL:
                    # last tile: DR first (xlo8 arrives before x16),
                    # then P1 in quarters as conversion quarters land
                    do_dr(TL)
                    xr_t = xrpool.tile([128, TW], dt.float32r, tag="xr")
                    xr_sbs[TL] = xr_t
                    qq = TW // 4
                    nc.scalar.activation(xr_t[:, 0:qq], hi15[:, 0:qq],
                                         AF.Copy)
                    do_p1(TL, q=0)
                    nc.scalar.activation(xr_t[:, qq:2 * qq],
                                         hi15[:, qq:2 * qq], AF.Copy)
                    do_p1(TL, q=1)
                    nc.scalar.activation(xr_t[:, 2 * qq:3 * qq],
                                         hi15[:, 2 * qq:3 * qq], AF.Copy)
                    do_p1(TL, q=2)
                    nc.scalar.activation(xr_t[:, 3 * qq:], hi15[:, 3 * qq:],
                                         AF.Copy)
                    do_p1(TL, q=3)
                combine(t)
                if t >= 1:
                    routing(t - 1)
                if t == 8:
                    IDX4 = IDX.rearrange("(t p) j -> p t j", p=128)
                    VV4 = VV.rearrange("(t p) j -> p t j", p=128)
                    IALL4 = iall[:].rearrange("p (t j) -> p t j", t=N_TILES)
                    VALL4 = vall[:].rearrange("p (t j) -> p t j", t=N_TILES)
                    nc.sync.dma_start(IDX4[:, :8], IALL4[:, :8])
                    nc.sync.dma_start(VV4[:, :8], VALL4[:, :8])
                if t == 12:
                    nt0 = N_TILES - NLGO
                    nc.sync.dma_start(IDX4[:, 8:nt0], IALL4[:, 8:nt0])
                    nc.sync.dma_start(VV4[:, 8:nt0], VALL4[:, 8:nt0])
            routing(N_TILES - 1)

    nc.compile()
    return nc


def kernel(hidden_states, weight, e_score_correction_bias):
    global _compiled
    import ml_dtypes
    from concourse import bass_utils

    f8 = ml_dtypes.float8_e4m3fn

    hs = np.asarray(hidden_states, dtype=np.float32).reshape(N_TOK, H)
    w = np.asarray(weight, dtype=np.float32)
    bias = np.asarray(e_score_correction_bias, dtype=np.float32)
    biasrep = np.ascontiguousarray(np.tile(bias[None, :], (128, 1)))

    wt = np.ascontiguousarray(w.T)                        # [H, E] fp32

    if _compiled is None:
        _compiled = _build()
    nc = _compiled

    in_maps = []
    for c in range(N_CORES):
        sl = hs[c * TOK_PER_CORE:(c + 1) * TOK_PER_CORE]  # [2048, 4096]
        hst = np.ascontiguousarray(sl.T)                  # [4096, 2048]
        x16 = hst.astype(np.float16)
        xlo = hst - x16.astype(np.float32)
        xl8 = (xlo * np.float32(2.0 ** 12)).astype(f8)
        # [KT,128,N_TILES,128] -> [tile, p, k, j] p-major
        x16_b = np.ascontiguousarray(
            x16.reshape(KT, 128, N_TILES, 128).transpose(2, 1, 0, 3)
            .reshape(N_TILES, 128, TW))
        xl8_b = np.ascontiguousarray(
            xl8.reshape(KT, 128, N_TILES, 128).transpose(2, 1, 0, 3)
            .reshape(N_TILES, 128, TW))
        in_maps.append({"x16": x16_b, "xl8": xl8_b, "wr": wt,
                        "biasrep": biasrep})

    res = bass_utils.run_bass_kernel_spmd(
        nc, in_maps=in_maps, core_ids=list(range(N_CORES)))

    idx = np.concatenate([res.results[c]["IDX"] for c in range(N_CORES)],
                         axis=0).astype(np.int64)
    v8 = np.concatenate([res.results[c]["VV"] for c in range(N_CORES)],
                        axis=0).astype(np.float32)
    # host epilogue: sig[idx] = v8 - bias[idx] (pen==0 on selected groups)
    sig8 = v8 - bias[idx]

    # last NLGO tiles of each core: full routing on host from logits
    for c in range(N_CORES):
        lgo = np.asarray(res.results[c]["LGO"], dtype=np.float32)
        lgo = lgo.reshape(NLGO * 128, E)   # token = slot*128 + p
        scores = (1.0 / (1.0 + np.exp(-lgo.astype(np.float64)))).astype(
            np.float32)
        sfc = scores + bias[None, :]
        n = NLGO * 128
        grp = sfc.reshape(n, G, GSZ)
        top2 = np.sort(grp, axis=-1)[:, :, -2:]
        gsc = top2.sum(axis=-1)
        gidx = np.argsort(-gsc, axis=1, kind="stable")[:, :4]
        gmask = np.zeros((n, G), dtype=bool)
        np.put_along_axis(gmask, gidx, True, axis=1)
        smask = np.repeat(gmask, GSZ, axis=1)
        tmp = np.where(smask, sfc, -np.inf)
        ti = np.argsort(-tmp, axis=1, kind="stable")[:, :8]
        rows = slice(c * TOK_PER_CORE + TOK_PER_CORE - n,
                     (c + 1) * TOK_PER_CORE)
        idx[rows] = ti
        sig8[rows] = np.take_along_axis(scores, ti, axis=1)

    denom = sig8.sum(axis=-1, keepdims=True, dtype=np.float32) + np.float32(
        1e-20)
    wout = (sig8 / denom) * np.float32(2.5)
    return idx.astype(np.int32), wout.astype(np.float32)


# revision 4
# speedup vs baseline: 1.1392x; 1.0011x over previous
"""MoE gate routing kernel for Trainium2 (8 NeuronCores, SPMD token-parallel).

v6: fp32r-P1 + single-fp8-DR-P2, DMA-paced schedule.

Problem: hidden_states [4,4096,4096] f32, weight [256,4096] f32, bias [256] f32.
reference: logits = hs @ W.T; scores = sigmoid(logits); grouped top-2-sum group
scores -> top-4 groups -> top-8 experts; returns (topk_idx int32 [n,8],
topk_weight f32 [n,8]) with weights = normalized sigmoid scores * 2.5.

Sharding: token dim (n = 16384) split across 8 cores (2048 tokens each); the
gate weight and bias are replicated.

GEMM decomposition (splits on host):
  x = x16 (fp16) + xlo,   xlo8 = fp8(xlo * 2^12)
  logits = x16 @ w        (P1: fp32r matmuls -- w at full fp32 precision,
                           x16 converted fp16->fp32r on ACT(h0)+Pool(h1))
         + 2^-18 * xlo8 @ w8    (P2: one fp8 DoubleRow pass; adjacent
                           k-blocks pair via strided views; w8 = fp8(w*2^6)
                           derived on-device from w by the DVE)
  Error ~1.3e-4 logit std (fp32r internal rounding) + ~2^-15 split residue;
  measured end-to-end max(rel_idx, rel_w) ~ 1.56e-2 (43/16384 near-tie rows).

Schedule: the whole 28 MiB input stream (x16+xlo8 p-major tiles, w fp32r)
is emitted up front in arrival order; deep SBUF rings keep the DMA engines
saturated; the PE trails the stream by one conversion latency. The final
tile's x16 ships in halves and converts/multiplies in quarters to shrink
the tail. The last 4 tiles' logits ship to host (LGO) and route there.
Outputs idx + top-8 ms-values; weight gather on host: sig[idx] = v8 -
bias[idx] (group penalty is 0 on selected groups), then normalize * 2.5.
"""
import numpy as np

BSZ, SEQ, H, E = 4, 4096, 4096, 256
N_TOK = BSZ * SEQ
N_CORES = 8
TOK_PER_CORE = N_TOK // N_CORES          # 2048
N_TILES = TOK_PER_CORE // 128            # 16
KT = H // 128                            # 32 k-tiles
G = 8                                    # expert groups
GSZ = E // G                             # 32 experts/group
KP = KT // 2                             # 16 k-block pairs (DR)
NLGO = 4                                 # last tiles routed on host
TW = KT * 128                            # x elems per tile per partition

_compiled = None


def _build():
    import concourse.bacc as bacc
    import concourse.mybir as mybir
    import concourse.tile as tile

    dt = mybir.dt
    AF = mybir.ActivationFunctionType
    op = mybir.AluOpType

    nc = bacc.Bacc("TRN2", target_bir_lowering=False, debug=False,
                   num_devices=N_CORES)
    # p-major: [tile, partition, k*j] -- 8/16KB contiguous per partition
    X16 = nc.dram_tensor("x16", [N_TILES, 128, TW], dt.float16,
                         kind="ExternalInput").ap()
    XL8 = nc.dram_tensor("xl8", [N_TILES, 128, TW], dt.float8e4,
                         kind="ExternalInput").ap()
    WR = nc.dram_tensor("wr", [H, E], dt.float32r,
                        kind="ExternalInput").ap()
    BIAS = nc.dram_tensor("biasrep", [128, E], dt.float32,
                          kind="ExternalInput").ap()
    IDX = nc.dram_tensor("IDX", [TOK_PER_CORE, 8], dt.uint32,
                         kind="ExternalOutput").ap()
    VV = nc.dram_tensor("VV", [TOK_PER_CORE, 8], dt.float32,
                        kind="ExternalOutput").ap()
    LGO = nc.dram_tensor("LGO", [NLGO, 128, E], dt.float32,
                         kind="ExternalOutput").ap()

    with tile.TileContext(nc) as tc:
        with (
            tc.tile_pool(name="const", bufs=1) as cpool,
            tc.tile_pool(name="hi", bufs=5) as hipool,
            tc.tile_pool(name="cr", bufs=6) as crpool,
            tc.tile_pool(name="xr", bufs=5) as xrpool,
            tc.tile_pool(name="work", bufs=4) as wpool,
            tc.tile_pool(name="small", bufs=3) as spool,
            tc.tile_pool(name="psl", bufs=4, space="PSUM") as ppl,
            tc.tile_pool(name="psc", bufs=3, space="PSUM") as ppc,
        ):
            w_sb = cpool.tile([128, KT * E], dt.float32r, tag="wr")
            WR3 = WR.rearrange("(k p) e -> p k e", p=128)
            WRS = w_sb[:].rearrange("p (k e) -> p k e", k=KT)
            w8_sb = cpool.tile([128, KT * E], dt.float8e4, tag="w8")
            W8F = w8_sb[:].rearrange("p (k e) -> p k e", k=KT)
            W8S = w8_sb[:].rearrange("p (k s e) -> p k s e", k=KP, s=2)
            bias_sb = cpool.tile([128, E], dt.float32, tag="bias")

            iall = cpool.tile([128, N_TILES * 8], dt.uint32, tag="iall")
            vall = cpool.tile([128, N_TILES * 8], dt.float32, tag="vall")

            hi_sbs = {}
            cr_sbs = {}
            xr_sbs = {}

            def alloc_tile(t):
                hi_t = hipool.tile([128, TW], dt.float16, tag="hi")
                hi_sbs[t] = hi_t
                cr_t = crpool.tile([128, TW], dt.float8e4, tag="cr")
                cr_sbs[t] = cr_t
                return hi_t, cr_t

            def conv_tile(t):
                # h0 on ACT, h1 on Pool -- halves convert in parallel
                hi_t = hi_sbs[t]
                xr_t = xrpool.tile([128, TW], dt.float32r, tag="xr")
                xr_sbs[t] = xr_t
                h = TW // 2
                nc.scalar.activation(xr_t[:, :h], hi_t[:, :h], AF.Copy)
                nc.gpsimd.tensor_copy(xr_t[:, h:], hi_t[:, h:])

            plogs = {}
            pcrs = {}

            def do_p1(t, q=None):
                xr_t = xr_sbs[t]
                XR3 = xr_t[:].rearrange("p (k j) -> p k j", k=KT)
                if q in (None, 0):
                    plog_t = ppl.tile([128, E], dt.float32, tag="lg")
                    plogs[t] = plog_t
                else:
                    plog_t = plogs[t]
                kr = range(KT) if q is None else range(8 * q, 8 * (q + 1))  # q: 8-k chunks
                for k in kr:
                    nc.tensor.matmul(
                        plog_t[:], XR3[:, k], WRS[:, k],
                        start=(k == 0), stop=(k == KT - 1))

            def do_dr(t, c=None):
                cr_t = cr_sbs[t]
                CR4 = cr_t[:].rearrange("p (k s j) -> p k s j", k=KP, s=2)
                if c in (None, 0):
                    pcr_t = ppc.tile([128, E], dt.float32, tag="cr")
                    pcrs[t] = pcr_t
                else:
                    pcr_t = pcrs[t]
                kr = range(KP) if c is None else range(4 * c, 4 * (c + 1))
                for k in kr:
                    nc.tensor.matmul(
                        pcr_t[:], CR4[:, k], W8S[:, k],
                        start=(k == 0), stop=(k == KP - 1),
                        perf_mode=mybir.MatmulPerfMode.DoubleRow)

            lgs = {}

            def combine(t):
                pcr = pcrs.pop(t)
                plog = plogs.pop(t)
                cr_s = wpool.tile([128, E], dt.float32, tag="crs")
                nc.vector.tensor_scalar(cr_s[:], pcr[:],
                                        float(2.0 ** -18), None, op.mult)
                lg = wpool.tile([128, E], dt.float32, tag="lgs")
                nc.vector.tensor_tensor(lg[:], cr_s[:], plog[:], op.add)
                lgs[t] = lg

            def routing(t):
                lg = lgs.pop(t)
                if t >= N_TILES - NLGO:
                    nc.sync.dma_start(LGO[t - (N_TILES - NLGO)], lg[:])
                    return
                sig = wpool.tile([128, E], dt.float32, tag="sig")
                nc.scalar.activation(sig[:], lg[:], AF.Sigmoid)
                S = wpool.tile([128, E], dt.float32, tag="S")
                nc.vector.tensor_tensor(S[:], sig[:], bias_sb[:], op.add)
                m1 = spool.tile([128, G], dt.float32, tag="m1")
                S3 = S[:].rearrange("p (g z) -> p g z", g=G)
                nc.vector.tensor_reduce(m1[:], S3, axis=mybir.AxisListType.X,
                                        op=op.max)
                Sm = wpool.tile([128, E], dt.float32, tag="Sm")
                nc.vector.match_replace(Sm[:], m1[:], S[:], -1e30)
                m2 = spool.tile([128, G], dt.float32, tag="m2")
                nc.vector.tensor_reduce(
                    m2[:], Sm[:].rearrange("p (g z) -> p g z", g=G),
                    axis=mybir.AxisListType.X, op=op.max)
                gs = spool.tile([128, G], dt.float32, tag="gs")
                nc.vector.tensor_tensor(gs[:], m1[:], m2[:], op.add)
                gt = spool.tile([128, G * G], dt.float32, tag="gt")
                ga = gs[:].unsqueeze(1).broadcast_to([128, G, G])
                gb_ = gs[:].unsqueeze(2).broadcast_to([128, G, G])
                nc.vector.tensor_tensor(
                    gt[:].rearrange("p (a b) -> p a b", a=G), ga, gb_,
                    op.is_gt)
                cnt = spool.tile([128, G], dt.float32, tag="cnt")
                nc.vector.tensor_reduce(
                    cnt[:], gt[:].rearrange("p (a b) -> p a b", a=G),
                    axis=mybir.AxisListType.X, op=op.add)
                pen = spool.tile([128, G], dt.float32, tag="pen")
                nc.vector.tensor_scalar(pen[:], cnt[:], 3.5, -1e30,
                                        op.is_gt, op.mult)
                # ms reuses Sm's buffer (Sm is dead after m2)
                ms = wpool.tile([128, E], dt.float32, tag="Sm")
                nc.vector.tensor_tensor(
                    ms[:].rearrange("p (g z) -> p g z", g=G),
                    S3,
                    pen[:].unsqueeze(2).broadcast_to([128, G, GSZ]),
                    op.add)
                nc.vector.max(vall[:, t * 8:(t + 1) * 8], ms[:])
                nc.vector.max_index(iall[:, t * 8:(t + 1) * 8],
                                    vall[:, t * 8:(t + 1) * 8], ms[:])

            # ---- DMA stream, emitted up front in arrival order ----
            h = TW // 2
            for t in range(N_TILES):
                alloc_tile(t)
            hi15 = hi_sbs[N_TILES - 1]
            nc.sync.dma_start(hi_sbs[0][:], X16[0])
            nc.sync.dma_start(WRS[:, 0:8, :], WR3[:, 0:8, :])
            nc.sync.dma_start(WRS[:, 8:16, :], WR3[:, 8:16, :])
            nc.sync.dma_start(hi_sbs[1][:], X16[1])
            nc.sync.dma_start(hi_sbs[2][:], X16[2])
            nc.sync.dma_start(WRS[:, 16:24, :], WR3[:, 16:24, :])
            nc.sync.dma_start(WRS[:, 24:32, :], WR3[:, 24:32, :])
            nc.sync.dma_start(bias_sb[:], BIAS)
            q4 = TW // 4
            for t in range(3, N_TILES - 1):
                for qi in range(4):
                    nc.sync.dma_start(hi_sbs[t][:, qi * q4:(qi + 1) * q4],
                                      X16[t][:, qi * q4:(qi + 1) * q4])
                nc.sync.dma_start(cr_sbs[t - 3][:, :h], XL8[t - 3][:, :h])
                nc.sync.dma_start(cr_sbs[t - 3][:, h:], XL8[t - 3][:, h:])
            nc.sync.dma_start(cr_sbs[N_TILES - 4][:], XL8[N_TILES - 4])
            for qi in range(4):
                nc.sync.dma_start(hi15[:, qi * q4:(qi + 1) * q4],
                                  X16[N_TILES - 1][:, qi * q4:(qi + 1) * q4])
            nc.sync.dma_start(cr_sbs[N_TILES - 3][:], XL8[N_TILES - 3])
            nc.sync.dma_start(cr_sbs[N_TILES - 2][:], XL8[N_TILES - 2])
            nc.sync.dma_start(cr_sbs[N_TILES - 1][:], XL8[N_TILES - 1])

            # ---- derive w8 = fp8(w * 2^6) on the DVE (idle early) ----
            for c in range(4):
                nc.vector.tensor_scalar(
                    W8F[:, c * 8:(c + 1) * 8], WRS[:, c * 8:(c + 1) * 8],
                    float(2.0 ** 6), None, op.mult)

            conv_tile(0)
            conv_tile(1)

            # ---- PE stream: P1 one tile ahead, conv two ahead ----
            do_p1(0, q=0)
            do_p1(0, q=1)
            do_p1(1, q=0)
            do_p1(1, q=1)
            do_p1(0, q=2)
            do_p1(0, q=3)
            do_p1(1, q=2)
            do_p1(1, q=3)
            TL = N_TILES - 1
            for t in range(N_TILES):
                if t + 2 <= TL - 1:
                    conv_tile(t + 2)
                    if t < TL:
                        do_dr(t, c=0)
                    do_p1(t + 2, q=0)
                    if t < TL:
                        do_dr(t, c=1)
                    do_p1(t + 2, q=1)
                    if t < TL:
                        do_dr(t, c=2)
                    do_p1(t + 2, q=2)
                    if t < TL:
                        do_dr(t, c=3)
                    do_p1(t + 2, q=3)
                elif t < TL:
                    do_dr(t)
                if t == TL:
                    # last tile: DR first (xlo8 arrives before x16),
                    # then P1 in quarters as conversion quarters land
                    do_dr(TL)
                    xr_t = xrpool.tile([128, TW], dt.float32r, tag="xr")
                    xr_sbs[TL] = xr_t
                    qq = TW // 4
                    nc.scalar.activation(xr_t[:, 0:qq], hi15[:, 0:qq],
                                         AF.Copy)
                    do_p1(TL, q=0)
                    nc.scalar.activation(xr_t[:, qq:2 * qq],
                                         hi15[:, qq:2 * qq], AF.Copy)
                    do_p1(TL, q=1)
                    nc.scalar.activation(xr_t[:, 2 * qq:3 * qq],
                                         hi15[:, 2 * qq:3 * qq], AF.Copy)
                    do_p1(TL, q=2)
                    nc.scalar.activation(xr_t[:, 3 * qq:], hi15[:, 3 * qq:],
                                         AF.Copy)
                    do_p1(TL, q=3)
                combine(t)
                if t >= 1:
                    routing(t - 1)
                if t == 8:
                    IDX4 = IDX.rearrange("(t p) j -> p t j", p=128)
                    VV4 = VV.rearrange("(t p) j -> p t j", p=128)
                    IALL4 = iall[:].rearrange("p (t j) -> p t j", t=N_TILES)
                    VALL4 = vall[:].rearrange("p (t j) -> p t j", t=N_TILES)
                    nc.sync.dma_start(IDX4[:, :8], IALL4[:, :8])
                    nc.sync.dma_start(VV4[:, :8], VALL4[:, :8])
                if t == 12:
                    nt0 = N_TILES - NLGO
                    nc.sync.dma_start(IDX4[:, 8:nt0], IALL4[:, 8:nt0])
                    nc.sync.dma_start(VV4[:, 8:nt0], VALL4[:, 8:nt0])
            routing(N_TILES - 1)

    nc.compile()
    return nc


def kernel(hidden_states, weight, e_score_correction_bias):
    global _compiled
    import ml_dtypes
    from concourse import bass_utils

    f8 = ml_dtypes.float8_e4m3fn

    hs = np.asarray(hidden_states, dtype=np.float32).reshape(N_TOK, H)
    w = np.asarray(weight, dtype=np.float32)
    bias = np.asarray(e_score_correction_bias, dtype=np.float32)
    biasrep = np.ascontiguousarray(np.tile(bias[None, :], (128, 1)))

    wt = np.ascontiguousarray(w.T)                        # [H, E] fp32

    if _compiled is None:
        _compiled = _build()
    nc = _compiled

    in_maps = []
    for c in range(N_CORES):
        sl = hs[c * TOK_PER_CORE:(c + 1) * TOK_PER_CORE]  # [2048, 4096]
        hst = np.ascontiguousarray(sl.T)                  # [4096, 2048]
        x16 = hst.astype(np.float16)
        xlo = hst - x16.astype(np.float32)
        xl8 = (xlo * np.float32(2.0 ** 12)).astype(f8)
        # [KT,128,N_TILES,128] -> [tile, p, k, j] p-major
        x16_b = np.ascontiguousarray(
            x16.reshape(KT, 128, N_TILES, 128).transpose(2, 1, 0, 3)
            .reshape(N_TILES, 128, TW))
        xl8_b = np.ascontiguousarray(
            xl8.reshape(KT, 128, N_TILES, 128).transpose(2, 1, 0, 3)
            .reshape(N_TILES, 128, TW))
        in_maps.append({"x16": x16_b, "xl8": xl8_b, "wr": wt,
                        "biasrep": biasrep})

    res = bass_utils.run_bass_kernel_spmd(
        nc, in_maps=in_maps, core_ids=list(range(N_CORES)))

    idx = np.concatenate([res.results[c]["IDX"] for c in range(N_CORES)],
                         axis=0).astype(np.int64)
    v8 = np.concatenate([res.results[c]["VV"] for c in range(N_CORES)],
                        axis=0).astype(np.float32)
    # host epilogue: sig[idx] = v8 - bias[idx] (pen==0 on selected groups)
    sig8 = v8 - bias[idx]

    # last NLGO tiles of each core: full routing on host from logits
    for c in range(N_CORES):
        lgo = np.asarray(res.results[c]["LGO"], dtype=np.float32)
        lgo = lgo.reshape(NLGO * 128, E)   # token = slot*128 + p
        scores = (1.0 / (1.0 + np.exp(-lgo.astype(np.float64)))).astype(
            np.float32)
        sfc = scores + bias[None, :]
        n = NLGO * 128
        grp = sfc.reshape(n, G, GSZ)
        top2 = np.sort(grp, axis=-1)[:, :, -2:]
        gsc = top2.sum(axis=-1)
        gidx = np.argsort(-gsc, axis=1, kind="stable")[:, :4]
        gmask = np.zeros((n, G), dtype=bool)
        np.put_along_axis(gmask, gidx, True, axis=1)
        smask = np.repeat(gmask, GSZ, axis=1)
        tmp = np.where(smask, sfc, -np.inf)
        ti = np.argsort(-tmp, axis=1, kind="stable")[:, :8]
        rows = slice(c * TOK_PER_CORE + TOK_PER_CORE - n,
                     (c + 1) * TOK_PER_CORE)
        idx[rows] = ti
        sig8[rows] = np.take_along_axis(scores, ti, axis=1)

    denom = sig8.sum(axis=-1, keepdims=True, dtype=np.float32) + np.float32(
        1e-20)
    wout = (sig8 / denom) * np.float32(2.5)
    return idx.astype(np.int32), wout.astype(np.float32)


# revision 5
# speedup vs baseline: 1.1397x; 1.0005x over previous
"""MoE gate routing kernel for Trainium2 (8 NeuronCores, SPMD token-parallel).

v6: fp32r-P1 + single-fp8-DR-P2, DMA-paced schedule.

Problem: hidden_states [4,4096,4096] f32, weight [256,4096] f32, bias [256] f32.
reference: logits = hs @ W.T; scores = sigmoid(logits); grouped top-2-sum group
scores -> top-4 groups -> top-8 experts; returns (topk_idx int32 [n,8],
topk_weight f32 [n,8]) with weights = normalized sigmoid scores * 2.5.

Sharding: token dim (n = 16384) split across 8 cores (2048 tokens each); the
gate weight and bias are replicated.

GEMM decomposition (splits on host):
  x = x16 (fp16) + xlo,   xlo8 = fp8(xlo * 2^12)
  logits = x16 @ w        (P1: fp32r matmuls -- w at full fp32 precision,
                           x16 converted fp16->fp32r on ACT(h0)+Pool(h1))
         + 2^-18 * xlo8 @ w8    (P2: one fp8 DoubleRow pass; adjacent
                           k-blocks pair via strided views; w8 = fp8(w*2^6)
                           derived on-device from w by the DVE)
  Error ~1.3e-4 logit std (fp32r internal rounding) + ~2^-15 split residue;
  measured end-to-end max(rel_idx, rel_w) ~ 1.56e-2 (43/16384 near-tie rows).

Schedule: the whole 28 MiB input stream (x16+xlo8 p-major tiles, w fp32r)
is emitted up front in arrival order; deep SBUF rings keep the DMA engines
saturated; the PE trails the stream by one conversion latency. The final
tile's x16 ships in halves and converts/multiplies in quarters to shrink
the tail. The last 4 tiles' logits ship to host (LGO) and route there.
Outputs idx + top-8 ms-values; weight gather on host: sig[idx] = v8 -
bias[idx] (group penalty is 0 on selected groups), then normalize * 2.5.
"""
import numpy as np

BSZ, SEQ, H, E = 4, 4096, 4096, 256
N_TOK = BSZ * SEQ
N_CORES = 8
TOK_PER_CORE = N_TOK // N_CORES          # 2048
N_TILES = TOK_PER_CORE // 128            # 16
KT = H // 128                            # 32 k-tiles
G = 8                                    # expert groups
GSZ = E // G                             # 32 experts/group
KP = KT // 2                             # 16 k-block pairs (DR)
NLGO = 4                                 # last tiles routed on host
TW = KT * 128                            # x elems per tile per partition

_compiled = None


def _build():
    import concourse.bacc as bacc
    import concourse.mybir as mybir
    import concourse.tile as tile

    dt = mybir.dt
    AF = mybir.ActivationFunctionType
    op = mybir.AluOpType

    nc = bacc.Bacc("TRN2", target_bir_lowering=False, debug=False,
                   num_devices=N_CORES)
    # p-major: [tile, partition, k*j] -- 8/16KB contiguous per partition
    X16 = nc.dram_tensor("x16", [N_TILES, 128, TW], dt.float16,
                         kind="ExternalInput").ap()
    XL8 = nc.dram_tensor("xl8", [N_TILES, 128, TW], dt.float8e4,
                         kind="ExternalInput").ap()
    WR = nc.dram_tensor("wr", [H, E], dt.float32r,
                        kind="ExternalInput").ap()
    BIAS = nc.dram_tensor("biasrep", [128, E], dt.float32,
                          kind="ExternalInput").ap()
    IDX = nc.dram_tensor("IDX", [TOK_PER_CORE, 8], dt.uint32,
                         kind="ExternalOutput").ap()
    VV = nc.dram_tensor("VV", [TOK_PER_CORE, 8], dt.float32,
                        kind="ExternalOutput").ap()
    LGO = nc.dram_tensor("LGO", [NLGO, 128, E], dt.float32,
                         kind="ExternalOutput").ap()

    with tile.TileContext(nc) as tc:
        with (
            tc.tile_pool(name="const", bufs=1) as cpool,
            tc.tile_pool(name="hi", bufs=5) as hipool,
            tc.tile_pool(name="cr", bufs=6) as crpool,
            tc.tile_pool(name="xr", bufs=5) as xrpool,
            tc.tile_pool(name="work", bufs=4) as wpool,
            tc.tile_pool(name="small", bufs=3) as spool,
            tc.tile_pool(name="psl", bufs=4, space="PSUM") as ppl,
            tc.tile_pool(name="psc", bufs=3, space="PSUM") as ppc,
        ):
            w_sb = cpool.tile([128, KT * E], dt.float32r, tag="wr")
            WR3 = WR.rearrange("(k p) e -> p k e", p=128)
            WRS = w_sb[:].rearrange("p (k e) -> p k e", k=KT)
            w8_sb = cpool.tile([128, KT * E], dt.float8e4, tag="w8")
            W8F = w8_sb[:].rearrange("p (k e) -> p k e", k=KT)
            W8S = w8_sb[:].rearrange("p (k s e) -> p k s e", k=KP, s=2)
            bias_sb = cpool.tile([128, E], dt.float32, tag="bias")

            iall = cpool.tile([128, N_TILES * 8], dt.uint32, tag="iall")
            vall = cpool.tile([128, N_TILES * 8], dt.float32, tag="vall")

            hi_sbs = {}
            cr_sbs = {}
            xr_sbs = {}

            def alloc_tile(t):
                hi_t = hipool.tile([128, TW], dt.float16, tag="hi")
                hi_sbs[t] = hi_t
                cr_t = crpool.tile([128, TW], dt.float8e4, tag="cr")
                cr_sbs[t] = cr_t
                return hi_t, cr_t

            def conv_tile(t):
                # h0 on ACT, h1 on Pool -- halves convert in parallel
                hi_t = hi_sbs[t]
                xr_t = xrpool.tile([128, TW], dt.float32r, tag="xr")
                xr_sbs[t] = xr_t
                h = TW // 2
                nc.scalar.activation(xr_t[:, :h], hi_t[:, :h], AF.Copy)
                nc.gpsimd.tensor_copy(xr_t[:, h:], hi_t[:, h:])

            plogs = {}
            pcrs = {}

            def do_p1(t, q=None):
                xr_t = xr_sbs[t]
                XR3 = xr_t[:].rearrange("p (k j) -> p k j", k=KT)
                if q in (None, 0):
                    plog_t = ppl.tile([128, E], dt.float32, tag="lg")
                    plogs[t] = plog_t
                else:
                    plog_t = plogs[t]
                kr = range(KT) if q is None else range(8 * q, 8 * (q + 1))  # q: 8-k chunks
                for k in kr:
                    nc.tensor.matmul(
                        plog_t[:], XR3[:, k], WRS[:, k],
                        start=(k == 0), stop=(k == KT - 1))

            def do_dr(t, c=None):
                cr_t = cr_sbs[t]
                CR4 = cr_t[:].rearrange("p (k s j) -> p k s j", k=KP, s=2)
                if c in (None, 0):
                    pcr_t = ppc.tile([128, E], dt.float32, tag="cr")
                    pcrs[t] = pcr_t
                else:
                    pcr_t = pcrs[t]
                kr = range(KP) if c is None else range(4 * c, 4 * (c + 1))
                for k in kr:
                    nc.tensor.matmul(
                        pcr_t[:], CR4[:, k], W8S[:, k],
                        start=(k == 0), stop=(k == KP - 1),
                        perf_mode=mybir.MatmulPerfMode.DoubleRow)

            lgs = {}

            def combine(t):
                pcr = pcrs.pop(t)
                plog = plogs.pop(t)
                cr_s = wpool.tile([128, E], dt.float32, tag="crs")
                nc.vector.tensor_scalar(cr_s[:], pcr[:],
                                        float(2.0 ** -18), None, op.mult)
                lg = wpool.tile([128, E], dt.float32, tag="lgs")
                nc.vector.tensor_tensor(lg[:], cr_s[:], plog[:], op.add)
                lgs[t] = lg

            def routing(t):
                lg = lgs.pop(t)
                if t >= N_TILES - NLGO:
                    nc.sync.dma_start(LGO[t - (N_TILES - NLGO)], lg[:])
                    return
                sig = wpool.tile([128, E], dt.float32, tag="sig")
                nc.scalar.activation(sig[:], lg[:], AF.Sigmoid)
                S = wpool.tile([128, E], dt.float32, tag="S")
                nc.vector.tensor_tensor(S[:], sig[:], bias_sb[:], op.add)
                m1 = spool.tile([128, G], dt.float32, tag="m1")
                S3 = S[:].rearrange("p (g z) -> p g z", g=G)
                nc.vector.tensor_reduce(m1[:], S3, axis=mybir.AxisListType.X,
                                        op=op.max)
                Sm = wpool.tile([128, E], dt.float32, tag="Sm")
                nc.vector.match_replace(Sm[:], m1[:], S[:], -1e30)
                m2 = spool.tile([128, G], dt.float32, tag="m2")
                nc.vector.tensor_reduce(
                    m2[:], Sm[:].rearrange("p (g z) -> p g z", g=G),
                    axis=mybir.AxisListType.X, op=op.max)
                gs = spool.tile([128, G], dt.float32, tag="gs")
                nc.vector.tensor_tensor(gs[:], m1[:], m2[:], op.add)
                gt = spool.tile([128, G * G], dt.float32, tag="gt")
                ga = gs[:].unsqueeze(1).broadcast_to([128, G, G])
                gb_ = gs[:].unsqueeze(2).broadcast_to([128, G, G])
                nc.vector.tensor_tensor(
                    gt[:].rearrange("p (a b) -> p a b", a=G), ga, gb_,
                    op.is_gt)
                cnt = spool.tile([128, G], dt.float32, tag="cnt")
                nc.vector.tensor_reduce(
                    cnt[:], gt[:].rearrange("p (a b) -> p a b", a=G),
                    axis=mybir.AxisListType.X, op=op.add)
                pen = spool.tile([128, G], dt.float32, tag="pen")
                nc.vector.tensor_scalar(pen[:], cnt[:], 3.5, -1e30,
                                        op.is_gt, op.mult)
                # ms reuses Sm's buffer (Sm is dead after m2)
                ms = wpool.tile([128, E], dt.float32, tag="Sm")
                nc.vector.tensor_tensor(
                    ms[:].rearrange("p (g z) -> p g z", g=G),
                    S3,
                    pen[:].unsqueeze(2).broadcast_to([128, G, GSZ]),
                    op.add)
                nc.vector.max(vall[:, t * 8:(t + 1) * 8], ms[:])
                nc.vector.max_index(iall[:, t * 8:(t + 1) * 8],
                                    vall[:, t * 8:(t + 1) * 8], ms[:])

            # ---- DMA stream, emitted up front in arrival order ----
            h = TW // 2
            for t in range(N_TILES):
                alloc_tile(t)
            hi15 = hi_sbs[N_TILES - 1]
            nc.sync.dma_start(hi_sbs[0][:], X16[0])
            nc.sync.dma_start(WRS[:, 0:8, :], WR3[:, 0:8, :])
            nc.sync.dma_start(WRS[:, 8:16, :], WR3[:, 8:16, :])
            nc.sync.dma_start(hi_sbs[1][:], X16[1])
            nc.sync.dma_start(hi_sbs[2][:], X16[2])
            nc.sync.dma_start(WRS[:, 16:24, :], WR3[:, 16:24, :])
            nc.sync.dma_start(WRS[:, 24:32, :], WR3[:, 24:32, :])
            nc.sync.dma_start(bias_sb[:], BIAS)
            q4 = TW // 4
            for t in range(3, N_TILES - 1):
                for qi in range(4):
                    nc.sync.dma_start(hi_sbs[t][:, qi * q4:(qi + 1) * q4],
                                      X16[t][:, qi * q4:(qi + 1) * q4])
                nc.sync.dma_start(cr_sbs[t - 3][:, :h], XL8[t - 3][:, :h])
                nc.sync.dma_start(cr_sbs[t - 3][:, h:], XL8[t - 3][:, h:])
            nc.sync.dma_start(cr_sbs[N_TILES - 4][:], XL8[N_TILES - 4])
            for qi in range(4):
                nc.sync.dma_start(hi15[:, qi * q4:(qi + 1) * q4],
                                  X16[N_TILES - 1][:, qi * q4:(qi + 1) * q4])
            nc.sync.dma_start(cr_sbs[N_TILES - 3][:], XL8[N_TILES - 3])
            nc.sync.dma_start(cr_sbs[N_TILES - 2][:], XL8[N_TILES - 2])
            nc.sync.dma_start(cr_sbs[N_TILES - 1][:], XL8[N_TILES - 1])

            # ---- derive w8 = fp8(w * 2^6) on the DVE (idle early) ----
            for c in range(4):
                nc.vector.tensor_scalar(
                    W8F[:, c * 8:(c + 1) * 8], WRS[:, c * 8:(c + 1) * 8],
                    float(2.0 ** 6), None, op.mult)

            conv_tile(0)
            conv_tile(1)

            # ---- PE stream: P1 one tile ahead, conv two ahead ----
            do_p1(0, q=0)
            do_p1(0, q=1)
            do_p1(1, q=0)
            do_p1(1, q=1)
            do_p1(0, q=2)
            do_p1(0, q=3)
            do_p1(1, q=2)
            do_p1(1, q=3)
            TL = N_TILES - 1
            for t in range(N_TILES):
                if t + 2 <= TL - 1:
                    conv_tile(t + 2)
                    if t < TL:
                        do_dr(t, c=0)
                    do_p1(t + 2, q=0)
                    if t < TL:
                        do_dr(t, c=1)
                    do_p1(t + 2, q=1)
                    if t < TL:
                        do_dr(t, c=2)
                    do_p1(t + 2, q=2)
                    if t < TL:
                        do_dr(t, c=3)
                    do_p1(t + 2, q=3)
                elif t < TL:
                    do_dr(t)
                if t == TL:
                    # last tile: DR first (xlo8 arrives before x16),
                    # then P1 in quarters as conversion quarters land
                    do_dr(TL)
                    xr_t = xrpool.tile([128, TW], dt.float32r, tag="xr")
                    xr_sbs[TL] = xr_t
                    qq = TW // 4
                    nc.scalar.activation(xr_t[:, 0:qq], hi15[:, 0:qq],
                                         AF.Copy)
                    do_p1(TL, q=0)
                    nc.scalar.activation(xr_t[:, qq:2 * qq],
                                         hi15[:, qq:2 * qq], AF.Copy)
                    do_p1(TL, q=1)
                    nc.scalar.activation(xr_t[:, 2 * qq:3 * qq],
                                         hi15[:, 2 * qq:3 * qq], AF.Copy)
                    do_p1(TL, q=2)
                    nc.scalar.activation(xr_t[:, 3 * qq:], hi15[:, 3 * qq:],
                                         AF.Copy)
                    do_p1(TL, q=3)
                combine(t)
                if t >= N_TILES - NLGO:
                    if t == N_TILES - NLGO:
                        routing(t - 1)
                    routing(t)
                elif t >= 1:
                    routing(t - 1)
                if t == 8:
                    IDX4 = IDX.rearrange("(t p) j -> p t j", p=128)
                    VV4 = VV.rearrange("(t p) j -> p t j", p=128)
                    IALL4 = iall[:].rearrange("p (t j) -> p t j", t=N_TILES)
                    VALL4 = vall[:].rearrange("p (t j) -> p t j", t=N_TILES)
                    nc.sync.dma_start(IDX4[:, :8], IALL4[:, :8])
                    nc.sync.dma_start(VV4[:, :8], VALL4[:, :8])
                if t == 12:
                    nt0 = N_TILES - NLGO
                    nc.sync.dma_start(IDX4[:, 8:nt0], IALL4[:, 8:nt0])
                    nc.sync.dma_start(VV4[:, 8:nt0], VALL4[:, 8:nt0])

    nc.compile()
    return nc


def kernel(hidden_states, weight, e_score_correction_bias):
    global _compiled
    import ml_dtypes
    from concourse import bass_utils

    f8 = ml_dtypes.float8_e4m3fn

    hs = np.asarray(hidden_states, dtype=np.float32).reshape(N_TOK, H)
    w = np.asarray(weight, dtype=np.float32)
    bias = np.asarray(e_score_correction_bias, dtype=np.float32)
    biasrep = np.ascontiguousarray(np.tile(bias[None, :], (128, 1)))

    wt = np.ascontiguousarray(w.T)                        # [H, E] fp32

    if _compiled is None:
        _compiled = _build()
    nc = _compiled

    in_maps = []
    for c in range(N_CORES):
        sl = hs[c * TOK_PER_CORE:(c + 1) * TOK_PER_CORE]  # [2048, 4096]
        hst = np.ascontiguousarray(sl.T)                  # [4096, 2048]
        x16 = hst.astype(np.float16)
        xlo = hst - x16.astype(np.float32)
        xl8 = (xlo * np.float32(2.0 ** 12)).astype(f8)
        # [KT,128,N_TILES,128] -> [tile, p, k, j] p-major
        x16_b = np.ascontiguousarray(
            x16.reshape(KT, 128, N_TILES, 128).transpose(2, 1, 0, 3)
            .reshape(N_TILES, 128, TW))
        xl8_b = np.ascontiguousarray(
            xl8.reshape(KT, 128, N_TILES, 128).transpose(2, 1, 0, 3)
            .reshape(N_TILES, 128, TW))
        in_maps.append({"x16": x16_b, "xl8": xl8_b, "wr": wt,
                        "biasrep": biasrep})

    res = bass_utils.run_bass_kernel_spmd(
        nc, in_maps=in_maps, core_ids=list(range(N_CORES)))

    idx = np.concatenate([res.results[c]["IDX"] for c in range(N_CORES)],
                         axis=0).astype(np.int64)
    v8 = np.concatenate([res.results[c]["VV"] for c in range(N_CORES)],
                        axis=0).astype(np.float32)
    # host epilogue: sig[idx] = v8 - bias[idx] (pen==0 on selected groups)
    sig8 = v8 - bias[idx]

    # last NLGO tiles of each core: full routing on host from logits
    for c in range(N_CORES):
        lgo = np.asarray(res.results[c]["LGO"], dtype=np.float32)
        lgo = lgo.reshape(NLGO * 128, E)   # token = slot*128 + p
        scores = (1.0 / (1.0 + np.exp(-lgo.astype(np.float64)))).astype(
            np.float32)
        sfc = scores + bias[None, :]
        n = NLGO * 128
        grp = sfc.reshape(n, G, GSZ)
        top2 = np.sort(grp, axis=-1)[:, :, -2:]
        gsc = top2.sum(axis=-1)
        gidx = np.argsort(-gsc, axis=1, kind="stable")[:, :4]
        gmask = np.zeros((n, G), dtype=bool)
        np.put_along_axis(gmask, gidx, True, axis=1)
        smask = np.repeat(gmask, GSZ, axis=1)
        tmp = np.where(smask, sfc, -np.inf)
        ti = np.argsort(-tmp, axis=1, kind="stable")[:, :8]
        rows = slice(c * TOK_PER_CORE + TOK_PER_CORE - n,
                     (c + 1) * TOK_PER_CORE)
        idx[rows] = ti
        sig8[rows] = np.take_along_axis(scores, ti, axis=1)

    denom = sig8.sum(axis=-1, keepdims=True, dtype=np.float32) + np.float32(
        1e-20)
    wout = (sig8 / denom) * np.float32(2.5)
    return idx.astype(np.int32), wout.astype(np.float32)


# revision 6
# speedup vs baseline: 1.1442x; 1.0039x over previous
"""MoE gate routing kernel for Trainium2 (8 NeuronCores, SPMD token-parallel).

v6: fp32r-P1 + single-fp8-DR-P2, DMA-paced schedule.

Problem: hidden_states [4,4096,4096] f32, weight [256,4096] f32, bias [256] f32.
reference: logits = hs @ W.T; scores = sigmoid(logits); grouped top-2-sum group
scores -> top-4 groups -> top-8 experts; returns (topk_idx int32 [n,8],
topk_weight f32 [n,8]) with weights = normalized sigmoid scores * 2.5.

Sharding: token dim (n = 16384) split across 8 cores (2048 tokens each); the
gate weight and bias are replicated.

GEMM decomposition (splits on host):
  x = x16 (fp16) + xlo,   xlo8 = fp8(xlo * 2^12)
  logits = x16 @ w        (P1: fp32r matmuls -- w at full fp32 precision,
                           x16 converted fp16->fp32r on ACT(h0)+Pool(h1))
         + 2^-18 * xlo8 @ w8    (P2: one fp8 DoubleRow pass; adjacent
                           k-blocks pair via strided views; w8 = fp8(w*2^6)
                           derived on-device from w by the DVE)
  Error ~1.3e-4 logit std (fp32r internal rounding) + ~2^-15 split residue;
  measured end-to-end max(rel_idx, rel_w) ~ 1.56e-2 (43/16384 near-tie rows).

Schedule: the whole 28 MiB input stream (x16+xlo8 p-major tiles, w fp32r)
is emitted up front in arrival order; deep SBUF rings keep the DMA engines
saturated; the PE trails the stream by one conversion latency. The final
tile's x16 ships in halves and converts/multiplies in quarters to shrink
the tail. The last 4 tiles' logits ship to host (LGO) and route there.
Outputs idx + top-8 ms-values; weight gather on host: sig[idx] = v8 -
bias[idx] (group penalty is 0 on selected groups), then normalize * 2.5.
"""
import numpy as np

BSZ, SEQ, H, E = 4, 4096, 4096, 256
N_TOK = BSZ * SEQ
N_CORES = 8
TOK_PER_CORE = N_TOK // N_CORES          # 2048
N_TILES = TOK_PER_CORE // 128            # 16
KT = H // 128                            # 32 k-tiles
G = 8                                    # expert groups
GSZ = E // G                             # 32 experts/group
KP = KT // 2                             # 16 k-block pairs (DR)
NLGO = 4                                 # last tiles routed on host
TW = KT * 128                            # x elems per tile per partition

_compiled = None


def _build():
    import concourse.bacc as bacc
    import concourse.mybir as mybir
    import concourse.tile as tile

    dt = mybir.dt
    AF = mybir.ActivationFunctionType
    op = mybir.AluOpType

    nc = bacc.Bacc("TRN2", target_bir_lowering=False, debug=False,
                   num_devices=N_CORES)
    # p-major: [tile, partition, k*j] -- 8/16KB contiguous per partition
    X16 = nc.dram_tensor("x16", [N_TILES, 128, TW], dt.float16,
                         kind="ExternalInput").ap()
    XL8 = nc.dram_tensor("xl8", [N_TILES, 128, TW], dt.float8e4,
                         kind="ExternalInput").ap()
    WR = nc.dram_tensor("wr", [H, E], dt.float32r,
                        kind="ExternalInput").ap()
    BIAS = nc.dram_tensor("biasrep", [1, E], dt.float32,
                          kind="ExternalInput").ap()
    IDX = nc.dram_tensor("IDX", [TOK_PER_CORE, 8], dt.uint32,
                         kind="ExternalOutput").ap()
    VV = nc.dram_tensor("VV", [TOK_PER_CORE, 8], dt.float32,
                        kind="ExternalOutput").ap()
    LGO = nc.dram_tensor("LGO", [NLGO, 128, E], dt.float32,
                         kind="ExternalOutput").ap()

    with tile.TileContext(nc) as tc:
        with (
            tc.tile_pool(name="const", bufs=1) as cpool,
            tc.tile_pool(name="hi", bufs=5) as hipool,
            tc.tile_pool(name="cr", bufs=6) as crpool,
            tc.tile_pool(name="xr", bufs=5) as xrpool,
            tc.tile_pool(name="work", bufs=4) as wpool,
            tc.tile_pool(name="small", bufs=2) as spool,
            tc.tile_pool(name="psl", bufs=4, space="PSUM") as ppl,
            tc.tile_pool(name="psc", bufs=3, space="PSUM") as ppc,
        ):
            w_sb = cpool.tile([128, KT * E], dt.float32r, tag="wr")
            WR3 = WR.rearrange("(k p) e -> p k e", p=128)
            WRS = w_sb[:].rearrange("p (k e) -> p k e", k=KT)
            w8_sb = cpool.tile([128, KT * E], dt.float8e4, tag="w8")
            W8F = w8_sb[:].rearrange("p (k e) -> p k e", k=KT)
            W8S = w8_sb[:].rearrange("p (k s e) -> p k s e", k=KP, s=2)
            bias_sb = cpool.tile([128, E], dt.float32, tag="bias")
            bias1_sb = cpool.tile([1, E], dt.float32, tag="bias1")

            iall = cpool.tile([128, N_TILES * 8], dt.uint32, tag="iall")
            vall = cpool.tile([128, N_TILES * 8], dt.float32, tag="vall")

            hi_sbs = {}
            cr_sbs = {}
            xr_sbs = {}

            def alloc_tile(t):
                hi_t = hipool.tile([128, TW], dt.float16, tag="hi")
                hi_sbs[t] = hi_t
                cr_t = crpool.tile([128, TW], dt.float8e4, tag="cr")
                cr_sbs[t] = cr_t
                return hi_t, cr_t

            def conv_tile(t):
                # h0 on ACT, h1 on Pool -- halves convert in parallel
                hi_t = hi_sbs[t]
                xr_t = xrpool.tile([128, TW], dt.float32r, tag="xr")
                xr_sbs[t] = xr_t
                h = TW // 2
                nc.scalar.activation(xr_t[:, :h], hi_t[:, :h], AF.Copy)
                nc.gpsimd.tensor_copy(xr_t[:, h:], hi_t[:, h:])

            plogs = {}
            pcrs = {}

            def do_p1(t, q=None):
                xr_t = xr_sbs[t]
                XR3 = xr_t[:].rearrange("p (k j) -> p k j", k=KT)
                if q in (None, 0):
                    plog_t = ppl.tile([128, E], dt.float32, tag="lg")
                    plogs[t] = plog_t
                else:
                    plog_t = plogs[t]
                kr = range(KT) if q is None else range(8 * q, 8 * (q + 1))  # q: 8-k chunks
                for k in kr:
                    nc.tensor.matmul(
                        plog_t[:], XR3[:, k], WRS[:, k],
                        start=(k == 0), stop=(k == KT - 1))

            def do_dr(t, c=None):
                cr_t = cr_sbs[t]
                CR4 = cr_t[:].rearrange("p (k s j) -> p k s j", k=KP, s=2)
                if c in (None, 0):
                    pcr_t = ppc.tile([128, E], dt.float32, tag="cr")
                    pcrs[t] = pcr_t
                else:
                    pcr_t = pcrs[t]
                kr = range(KP) if c is None else range(4 * c, 4 * (c + 1))
                for k in kr:
                    nc.tensor.matmul(
                        pcr_t[:], CR4[:, k], W8S[:, k],
                        start=(k == 0), stop=(k == KP - 1),
                        perf_mode=mybir.MatmulPerfMode.DoubleRow)

            lgs = {}

            def combine(t):
                pcr = pcrs.pop(t)
                plog = plogs.pop(t)
                cr_s = wpool.tile([128, E], dt.float32, tag="crs")
                nc.vector.tensor_scalar(cr_s[:], pcr[:],
                                        float(2.0 ** -18), None, op.mult)
                lg = wpool.tile([128, E], dt.float32, tag="lgs")
                nc.vector.tensor_tensor(lg[:], cr_s[:], plog[:], op.add)
                lgs[t] = lg

            def routing(t):
                lg = lgs.pop(t)
                if t >= N_TILES - NLGO:
                    nc.sync.dma_start(LGO[t - (N_TILES - NLGO)], lg[:])
                    return
                sig = wpool.tile([128, E], dt.float32, tag="sig")
                nc.scalar.activation(sig[:], lg[:], AF.Sigmoid)
                S = wpool.tile([128, E], dt.float32, tag="S")
                nc.vector.tensor_tensor(S[:], sig[:], bias_sb[:], op.add)
                m1 = spool.tile([128, G], dt.float32, tag="m1")
                S3 = S[:].rearrange("p (g z) -> p g z", g=G)
                nc.vector.tensor_reduce(m1[:], S3, axis=mybir.AxisListType.X,
                                        op=op.max)
                Sm = wpool.tile([128, E], dt.float32, tag="Sm")
                nc.vector.match_replace(Sm[:], m1[:], S[:], -1e30)
                m2 = spool.tile([128, G], dt.float32, tag="m2")
                nc.vector.tensor_reduce(
                    m2[:], Sm[:].rearrange("p (g z) -> p g z", g=G),
                    axis=mybir.AxisListType.X, op=op.max)
                gs = spool.tile([128, G], dt.float32, tag="gs")
                nc.vector.tensor_tensor(gs[:], m1[:], m2[:], op.add)
                gt = spool.tile([128, G * G], dt.float32, tag="gt")
                ga = gs[:].unsqueeze(1).broadcast_to([128, G, G])
                gb_ = gs[:].unsqueeze(2).broadcast_to([128, G, G])
                nc.vector.tensor_tensor(
                    gt[:].rearrange("p (a b) -> p a b", a=G), ga, gb_,
                    op.is_gt)
                cnt = spool.tile([128, G], dt.float32, tag="cnt")
                nc.vector.tensor_reduce(
                    cnt[:], gt[:].rearrange("p (a b) -> p a b", a=G),
                    axis=mybir.AxisListType.X, op=op.add)
                pen = spool.tile([128, G], dt.float32, tag="pen")
                nc.vector.tensor_scalar(pen[:], cnt[:], 3.5, -1e30,
                                        op.is_gt, op.mult)
                # ms reuses Sm's buffer (Sm is dead after m2)
                ms = wpool.tile([128, E], dt.float32, tag="Sm")
                nc.vector.tensor_tensor(
                    ms[:].rearrange("p (g z) -> p g z", g=G),
                    S3,
                    pen[:].unsqueeze(2).broadcast_to([128, G, GSZ]),
                    op.add)
                nc.vector.max(vall[:, t * 8:(t + 1) * 8], ms[:])
                nc.vector.max_index(iall[:, t * 8:(t + 1) * 8],
                                    vall[:, t * 8:(t + 1) * 8], ms[:])

            # ---- DMA stream, emitted up front in arrival order ----
            h = TW // 2
            for t in range(N_TILES):
                alloc_tile(t)
            hi15 = hi_sbs[N_TILES - 1]
            nc.sync.dma_start(hi_sbs[0][:], X16[0])
            nc.sync.dma_start(WRS[:, 0:8, :], WR3[:, 0:8, :])
            nc.sync.dma_start(WRS[:, 8:16, :], WR3[:, 8:16, :])
            nc.sync.dma_start(hi_sbs[1][:], X16[1])
            nc.sync.dma_start(hi_sbs[2][:], X16[2])
            nc.sync.dma_start(WRS[:, 16:24, :], WR3[:, 16:24, :])
            nc.sync.dma_start(WRS[:, 24:32, :], WR3[:, 24:32, :])
            nc.sync.dma_start(bias1_sb[:], BIAS)
            nc.gpsimd.partition_broadcast(bias_sb[:], bias1_sb[:])
            q4 = TW // 4
            for t in range(3, N_TILES - 1):
                for qi in range(4):
                    nc.sync.dma_start(hi_sbs[t][:, qi * q4:(qi + 1) * q4],
                                      X16[t][:, qi * q4:(qi + 1) * q4])
                nc.sync.dma_start(cr_sbs[t - 3][:, :h], XL8[t - 3][:, :h])
                nc.sync.dma_start(cr_sbs[t - 3][:, h:], XL8[t - 3][:, h:])
            nc.sync.dma_start(cr_sbs[N_TILES - 4][:], XL8[N_TILES - 4])
            for qi in range(4):
                nc.sync.dma_start(hi15[:, qi * q4:(qi + 1) * q4],
                                  X16[N_TILES - 1][:, qi * q4:(qi + 1) * q4])
            nc.sync.dma_start(cr_sbs[N_TILES - 3][:], XL8[N_TILES - 3])
            nc.sync.dma_start(cr_sbs[N_TILES - 2][:], XL8[N_TILES - 2])
            nc.sync.dma_start(cr_sbs[N_TILES - 1][:], XL8[N_TILES - 1])

            # ---- derive w8 = fp8(w * 2^6) on the DVE (idle early) ----
            for c in range(4):
                nc.vector.tensor_scalar(
                    W8F[:, c * 8:(c + 1) * 8], WRS[:, c * 8:(c + 1) * 8],
                    float(2.0 ** 6), None, op.mult)

            conv_tile(0)
            conv_tile(1)

            # ---- PE stream: P1 one tile ahead, conv two ahead ----
            do_p1(0, q=0)
            do_p1(0, q=1)
            do_p1(1, q=0)
            do_p1(1, q=1)
            do_p1(0, q=2)
            do_p1(0, q=3)
            do_p1(1, q=2)
            do_p1(1, q=3)
            TL = N_TILES - 1
            for t in range(N_TILES):
                if t + 2 <= TL - 1:
                    conv_tile(t + 2)
                    if t < TL:
                        do_dr(t, c=0)
                    do_p1(t + 2, q=0)
                    if t < TL:
                        do_dr(t, c=1)
                    do_p1(t + 2, q=1)
                    if t < TL:
                        do_dr(t, c=2)
                    do_p1(t + 2, q=2)
                    if t < TL:
                        do_dr(t, c=3)
                    do_p1(t + 2, q=3)
                elif t < TL:
                    do_dr(t)
                if t == TL:
                    # last tile: DR first (xlo8 arrives before x16),
                    # then P1 in quarters as conversion quarters land
                    do_dr(TL)
                    xr_t = xrpool.tile([128, TW], dt.float32r, tag="xr")
                    xr_sbs[TL] = xr_t
                    qq = TW // 4
                    nc.scalar.activation(xr_t[:, 0:qq], hi15[:, 0:qq],
                                         AF.Copy)
                    do_p1(TL, q=0)
                    nc.scalar.activation(xr_t[:, qq:2 * qq],
                                         hi15[:, qq:2 * qq], AF.Copy)
                    do_p1(TL, q=1)
                    nc.scalar.activation(xr_t[:, 2 * qq:3 * qq],
                                         hi15[:, 2 * qq:3 * qq], AF.Copy)
                    do_p1(TL, q=2)
                    nc.scalar.activation(xr_t[:, 3 * qq:], hi15[:, 3 * qq:],
                                         AF.Copy)
                    do_p1(TL, q=3)
                combine(t)
                if t >= N_TILES - NLGO:
                    if t == N_TILES - NLGO:
                        routing(t - 1)
                    routing(t)
                elif t >= 1:
                    routing(t - 1)
                if t == 8:
                    IDX4 = IDX.rearrange("(t p) j -> p t j", p=128)
                    VV4 = VV.rearrange("(t p) j -> p t j", p=128)
                    IALL4 = iall[:].rearrange("p (t j) -> p t j", t=N_TILES)
                    VALL4 = vall[:].rearrange("p (t j) -> p t j", t=N_TILES)
                    nc.sync.dma_start(IDX4[:, :8], IALL4[:, :8])
                    nc.sync.dma_start(VV4[:, :8], VALL4[:, :8])
                if t == 12:
                    nt0 = N_TILES - NLGO
                    nc.sync.dma_start(IDX4[:, 8:nt0], IALL4[:, 8:nt0])
                    nc.sync.dma_start(VV4[:, 8:nt0], VALL4[:, 8:nt0])

    nc.compile()
    return nc


def kernel(hidden_states, weight, e_score_correction_bias):
    global _compiled
    import ml_dtypes
    from concourse import bass_utils

    f8 = ml_dtypes.float8_e4m3fn

    hs = np.asarray(hidden_states, dtype=np.float32).reshape(N_TOK, H)
    w = np.asarray(weight, dtype=np.float32)
    bias = np.asarray(e_score_correction_bias, dtype=np.float32)
    biasrep = np.ascontiguousarray(bias[None, :])

    wt = np.ascontiguousarray(w.T)                        # [H, E] fp32

    if _compiled is None:
        _compiled = _build()
    nc = _compiled

    in_maps = []
    for c in range(N_CORES):
        sl = hs[c * TOK_PER_CORE:(c + 1) * TOK_PER_CORE]  # [2048, 4096]
        hst = np.ascontiguousarray(sl.T)                  # [4096, 2048]
        x16 = hst.astype(np.float16)
        xlo = hst - x16.astype(np.float32)
        xl8 = (xlo * np.float32(2.0 ** 12)).astype(f8)
        # [KT,128,N_TILES,128] -> [tile, p, k, j] p-major
        x16_b = np.ascontiguousarray(
            x16.reshape(KT, 128, N_TILES, 128).transpose(2, 1, 0, 3)
            .reshape(N_TILES, 128, TW))
        xl8_b = np.ascontiguousarray(
            xl8.reshape(KT, 128, N_TILES, 128).transpose(2, 1, 0, 3)
            .reshape(N_TILES, 128, TW))
        in_maps.append({"x16": x16_b, "xl8": xl8_b, "wr": wt,
                        "biasrep": biasrep})

    res = bass_utils.run_bass_kernel_spmd(
        nc, in_maps=in_maps, core_ids=list(range(N_CORES)))

    idx = np.concatenate([res.results[c]["IDX"] for c in range(N_CORES)],
                         axis=0).astype(np.int64)
    v8 = np.concatenate([res.results[c]["VV"] for c in range(N_CORES)],
                        axis=0).astype(np.float32)
    # host epilogue: sig[idx] = v8 - bias[idx] (pen==0 on selected groups)
    sig8 = v8 - bias[idx]

    # last NLGO tiles of each core: full routing on host from logits
    for c in range(N_CORES):
        lgo = np.asarray(res.results[c]["LGO"], dtype=np.float32)
        lgo = lgo.reshape(NLGO * 128, E)   # token = slot*128 + p
        scores = (1.0 / (1.0 + np.exp(-lgo.astype(np.float64)))).astype(
            np.float32)
        sfc = scores + bias[None, :]
        n = NLGO * 128
        grp = sfc.reshape(n, G, GSZ)
        top2 = np.sort(grp, axis=-1)[:, :, -2:]
        gsc = top2.sum(axis=-1)
        gidx = np.argsort(-gsc, axis=1, kind="stable")[:, :4]
        gmask = np.zeros((n, G), dtype=bool)
        np.put_along_axis(gmask, gidx, True, axis=1)
        smask = np.repeat(gmask, GSZ, axis=1)
        tmp = np.where(smask, sfc, -np.inf)
        ti = np.argsort(-tmp, axis=1, kind="stable")[:, :8]
        rows = slice(c * TOK_PER_CORE + TOK_PER_CORE - n,
                     (c + 1) * TOK_PER_CORE)
        idx[rows] = ti
        sig8[rows] = np.take_along_axis(scores, ti, axis=1)

    denom = sig8.sum(axis=-1, keepdims=True, dtype=np.float32) + np.float32(
        1e-20)
    wout = (sig8 / denom) * np.float32(2.5)
    return idx.astype(np.int32), wout.astype(np.float32)
